# revision 23
# baseline (speedup 1.0000x reference)
"""Trainium2 Bass kernel for nn_CustomMultiheadAttention (linear attention with
low-rank QKV projections), SPMD over 8 NeuronCores.

Sharding: (batch, seq-half) -> core.  Core c handles batch c//2, sequence half
c%2 (2048 tokens).  Pairwise AllReduce of kv/k_sum between the two cores
sharing a batch.

Host-side prep: x and all weights are pre-transposed and cast to bf16 in
numpy so the device sees feature-major operands directly (no PE transposes,
no cast copies).  Bias adds are folded into the matmul accumulation groups as
K=1 ones-vector matmuls.  Attention denominators are accumulated for all 16
heads at once via a [128,16] selection-matrix matmul, inverted with one
reciprocal_approx_fast per chunk, and broadcast back with a K=16 pattern
matmul; head pairs share one [128,CH] PSUM tile so the divide is a single
tensor_tensor per pair.

elu(z)+1 is computed as exp(min(z,0)) + relu(z)  (exact).
"""

import numpy as np
import ml_dtypes

import concourse.bass as bass
import concourse.tile as tile
from concourse import bacc, mybir
from concourse.bass_utils import run_bass_kernel_spmd

F32 = mybir.dt.float32
BF16 = mybir.dt.bfloat16

B, S, E, H, R = 4, 4096, 1024, 16, 512
D = E // H  # 64
N_CORES = 8


def build_nc(T, n_cores, groups, debug_dump=False):
    """Build the SPMD bass kernel for T tokens per core."""
    CH = min(512, T)        # tokens per chunk
    NCH = T // CH           # chunks
    TB = CH // 128          # 128-token blocks per chunk
    EC = E // 128           # 8 feature chunks of 128
    RC = R // 128           # 4
    FC5 = E // 512          # 2 (512-wide f chunks)
    HP = H // 2             # head pairs
    hpf = 512 // D          # heads per 512-wide chunk = 8

    nc = bacc.Bacc("TRN2", target_bir_lowering=False, debug=False,
                   num_devices=n_cores)

    xq = nc.declare_dram_parameter("xq", [E, T], BF16, isOutput=False).ap()
    xk = nc.declare_dram_parameter("xk", [E, T], BF16, isOutput=False).ap()
    xv = nc.declare_dram_parameter("xv", [E, T], BF16, isOutput=False).ap()
    # host passes W.T for every weight
    qdT = nc.declare_dram_parameter("qdT", [E, R], BF16, isOutput=False).ap()
    kdT = nc.declare_dram_parameter("kdT", [E, R], BF16, isOutput=False).ap()
    vdT = nc.declare_dram_parameter("vdT", [E, R], BF16, isOutput=False).ap()
    quT = nc.declare_dram_parameter("quT", [R, E], BF16, isOutput=False).ap()
    kuT = nc.declare_dram_parameter("kuT", [R, E], BF16, isOutput=False).ap()
    vuT = nc.declare_dram_parameter("vuT", [R, E], BF16, isOutput=False).ap()
    owT = nc.declare_dram_parameter("owT", [E, E], BF16, isOutput=False).ap()
    qu_b = nc.declare_dram_parameter("qu_b", [E], BF16, isOutput=False).ap()
    ku_b = nc.declare_dram_parameter("ku_b", [E], BF16, isOutput=False).ap()
    vu_b = nc.declare_dram_parameter("vu_b", [E], BF16, isOutput=False).ap()
    out_b = nc.declare_dram_parameter("out_b", [E], BF16, isOutput=False).ap()
    # host-built broadcast pattern: ppat[h, hp*128+p] = 1 iff head h covers
    # partition p of pair hp (p<64 -> even head, p>=64 -> odd head)
    ppat_d = nc.declare_dram_parameter("ppat", [16, HP * 128], BF16,
                                       isOutput=False).ap()
    y = nc.declare_dram_parameter("y", [T, E], F32, isOutput=True).ap()

    # kv | k_sum buffers: head pair hp -> partitions [0:64] = head 2*hp,
    # [64:128] = head 2*hp+1
    cc_in = nc.dram_tensor("cc_in", [128, HP, D + 1], F32).ap()
    cc_out = nc.dram_tensor("cc_out", [128, HP, D + 1], F32).ap()

    dbg = {}
    if debug_dump:
        for nm, shp in (("qd", [128, RC, T]), ("ktm", [128, TB, H, D + 1]),
                        ("vtm", [128, TB, H, D + 1]), ("kvacc", [128, HP, D + 1]),
                        ("kvred", [128, HP, D + 1]), ("qfm", [128, EC, CH]),
                        ("den", [16, CH]), ("rec", [16, CH]),
                        ("nd0", [128, CH]), ("bc0", [128, CH]),
                        ("att", [128, EC, CH])):
            dbg[nm] = nc.declare_dram_parameter(f"dbg_{nm}", shp, F32,
                                                isOutput=True).ap()

    Exp = mybir.ActivationFunctionType.Exp
    Relu = mybir.ActivationFunctionType.Relu
    Copy = mybir.ActivationFunctionType.Copy

    with tile.TileContext(nc) as tc:
        with (
            tc.tile_pool(name="const", bufs=1) as const,
            tc.tile_pool(name="persist", bufs=1) as persist,
            tc.tile_pool(name="w2a", bufs=1) as w2a,
        ):
            ones_row = const.tile([1, 512], BF16)
            nc.vector.memset(ones_row[:], 1.0)

            # broadcast pattern: bc[p,t] = sum_h ppat[h,hp,p] * rec[h,t]
            ppat = const.tile([16, HP, 128], BF16)
            nc.sync.dma_start(
                out=ppat[:], in_=ppat_d.rearrange("h (c p) -> h c p", p=128))

            qu_brow = const.tile([1, E], BF16, tag="qub")
            ku_brow = const.tile([1, E], BF16, tag="kub")
            vu_brow = const.tile([1, E], BF16, tag="vub")
            ob_row = const.tile([1, E], BF16, tag="ob")
            for t_, a_ in ((qu_brow, qu_b), (ku_brow, ku_b),
                           (vu_brow, vu_b), (ob_row, out_b)):
                nc.sync.dma_start(out=t_[:],
                                  in_=a_.rearrange("(o f) -> o f", o=1))

            qd_all = persist.tile([128, RC, T], BF16)     # Qd feature-major
            kv_acc = persist.tile([128, HP, D + 1], F32)  # kv | k_sum

            # ---------------- Phase 1 ----------------
            with (
                tc.tile_pool(name="wkv", bufs=1) as wkv,
            ):
                kdT_sb = wkv.tile([128, EC, R], BF16, tag="kdT")
                vdT_sb = wkv.tile([128, EC, R], BF16, tag="vdT")
                kuT_sb = wkv.tile([128, RC, E], BF16, tag="kuT")
                vuT_sb = wkv.tile([128, RC, E], BF16, tag="vuT")
                quT_sb = w2a.tile([128, RC, E], BF16, tag="quT")
                nc.sync.dma_start(
                    out=kdT_sb[:], in_=kdT.rearrange("(c p) r -> p c r", p=128))
                nc.sync.dma_start(
                    out=vdT_sb[:], in_=vdT.rearrange("(c p) r -> p c r", p=128))
                nc.sync.dma_start(
                    out=kuT_sb[:], in_=kuT.rearrange("(c p) e -> p c e", p=128))
                nc.sync.dma_start(
                    out=vuT_sb[:], in_=vuT.rearrange("(c p) e -> p c e", p=128))
                nc.sync.dma_start(
                    out=quT_sb[:], in_=quT.rearrange("(c p) e -> p c e", p=128))

                with (
                    tc.tile_pool(name="xfm", bufs=2) as xfmp,
                    tc.tile_pool(name="dpsum", bufs=2, space="PSUM") as dpsum,
                ):
                    def load_x_fm(x_ap, c):
                        xfm = xfmp.tile([128, EC, CH], BF16, tag="xfm")
                        nc.sync.dma_start(
                            out=xfm[:],
                            in_=x_ap.rearrange("(f p) t -> p f t", p=128)
                                    [:, :, c * CH:(c + 1) * CH])
                        return xfm

                    def down_proj(xfm, dwT, dst, dst_sl):
                        # dst[:, rb, dst_sl] = (W_down @ x^T) feature-major
                        for rb in range(RC):
                            ps = dpsum.tile([128, CH], F32, tag="dps")
                            for ec in range(EC):
                                nc.tensor.matmul(
                                    ps[:], dwT[:, ec, rb * 128:(rb + 1) * 128],
                                    xfm[:, ec, :],
                                    start=(ec == 0), stop=(ec == EC - 1))
                            nc.vector.tensor_copy(dst[:, rb, dst_sl], ps[:])

                    # --- 1a: q down-projection ---
                    with tc.tile_pool(name="w1q", bufs=1) as w1q:
                        qdT_sb = w1q.tile([128, EC, R], BF16, tag="qdT")
                        nc.sync.dma_start(
                            out=qdT_sb[:],
                            in_=qdT.rearrange("(c p) r -> p c r", p=128))
                        for c in range(NCH):
                            xfm = load_x_fm(xq, c)
                            down_proj(xfm, qdT_sb, qd_all,
                                      bass.ds(c * CH, CH))
                        if debug_dump:
                            nc.gpsimd.dma_start(out=dbg["qd"][:],
                                                in_=qd_all[:])

                    # --- 1b: k/v down+up, elu, kv accumulation ---
                    with (
                        tc.tile_pool(name="dd", bufs=2) as ddp,
                        tc.tile_pool(name="upsum", bufs=2,
                                     space="PSUM") as upsum,
                        tc.tile_pool(name="ktm", bufs=1) as ktmp,
                        tc.tile_pool(name="vtm", bufs=1) as vtmp,
                        tc.tile_pool(name="elu1", bufs=2) as elu1,
                        tc.tile_pool(name="kvps", bufs=4,
                                     space="PSUM") as kvps,
                    ):

                        def up_k(dd, ktm):
                            for tb in range(TB):
                                for fc in range(FC5):
                                    ps = upsum.tile([128, 512], F32,
                                                    tag="ups")
                                    for rc in range(RC):
                                        nc.tensor.matmul(
                                            ps[:],
                                            dd[:, rc,
                                               tb * 128:(tb + 1) * 128],
                                            kuT_sb[:, rc,
                                                   fc * 512:(fc + 1) * 512],
                                            start=(rc == 0), stop=False)
                                    nc.tensor.matmul(
                                        ps[:], ones_row[:, 0:128],
                                        ku_brow[:, fc * 512:(fc + 1) * 512],
                                        start=False, stop=True)
                                    m = elu1.tile([128, 512], F32, tag="m")
                                    nc.vector.tensor_scalar_min(
                                        m[:], ps[:], 0.0)
                                    e = elu1.tile([128, 512], F32, tag="e")
                                    nc.scalar.activation(e[:], m[:], Exp)
                                    r = elu1.tile([128, 512], F32, tag="r")
                                    nc.scalar.activation(r[:], ps[:], Relu)
                                    dsl = ktm[:, tb,
                                              fc * hpf:(fc + 1) * hpf, 0:D]
                                    nc.vector.tensor_add(
                                        dsl,
                                        e[:].rearrange("p (h f) -> p h f",
                                                       h=hpf),
                                        r[:].rearrange("p (h f) -> p h f",
                                                       h=hpf))

                        def up_v(dd, vtm):
                            for tb in range(TB):
                                for fc in range(FC5):
                                    ps = upsum.tile([128, 512], F32,
                                                    tag="ups")
                                    for rc in range(RC):
                                        nc.tensor.matmul(
                                            ps[:],
                                            dd[:, rc,
                                               tb * 128:(tb + 1) * 128],
                                            vuT_sb[:, rc,
                                                   fc * 512:(fc + 1) * 512],
                                            start=(rc == 0), stop=False)
                                    nc.tensor.matmul(
                                        ps[:], ones_row[:, 0:128],
                                        vu_brow[:, fc * 512:(fc + 1) * 512],
                                        start=False, stop=True)
                                    dsl = vtm[:, tb,
                                              fc * hpf:(fc + 1) * hpf, 0:D]
                                    nc.scalar.activation(
                                        dsl,
                                        ps[:].rearrange("p (h f) -> p h f",
                                                        h=hpf), Copy)

                        for c in range(NCH):
                            xfm = load_x_fm(xk, c)
                            ddk = ddp.tile([128, RC, CH], BF16, tag="dd")
                            down_proj(xfm, kdT_sb, ddk, slice(None))
                            ktm = ktmp.tile([128, TB, H, D + 1], BF16,
                                            tag="ktm")
                            up_k(ddk, ktm)

                            xfm = load_x_fm(xv, c)
                            ddv = ddp.tile([128, RC, CH], BF16, tag="dd")
                            down_proj(xfm, vdT_sb, ddv, slice(None))
                            vtm = vtmp.tile([128, TB, H, D + 1], BF16,
                                            tag="vtm")
                            nc.vector.memset(vtm[:, :, :, D:D + 1], 1.0)
                            up_v(ddv, vtm)

                            if debug_dump and c == 0:
                                nc.gpsimd.dma_start(out=dbg["ktm"][:],
                                                    in_=ktm[:])
                                nc.gpsimd.dma_start(out=dbg["vtm"][:],
                                                    in_=vtm[:])

                            for h in range(H):
                                base = 64 * (h % 2)
                                pkv = kvps.tile([64, D + 1], F32, tag="kvps")
                                for tb in range(TB):
                                    nc.tensor.matmul(
                                        pkv[:], ktm[:, tb, h, 0:D],
                                        vtm[:, tb, h, 0:D + 1],
                                        start=(tb == 0), stop=(tb == TB - 1))
                                acc_sl = kv_acc[base:base + 64, h // 2, :]
                                if c == 0:
                                    nc.vector.tensor_copy(acc_sl, pkv[:])
                                else:
                                    nc.vector.tensor_add(acc_sl, acc_sl,
                                                         pkv[:])
                        if debug_dump:
                            nc.sync.dma_start(out=dbg["kvacc"][:],
                                              in_=kv_acc[:])

            # ---------------- AllReduce kv across the batch pair ----------
            nc.sync.dma_start(out=cc_in[:], in_=kv_acc[:])
            nc.gpsimd.collective_compute(
                "AllReduce", mybir.AluOpType.add,
                ins=[cc_in[:]], outs=[cc_out[:]],
                replica_groups=groups)

            # ---------------- Phase 2 ----------------
            with (
                tc.tile_pool(name="w2", bufs=1) as w2,
                tc.tile_pool(name="kvx", bufs=1) as kvx,
                tc.tile_pool(name="qps", bufs=2, space="PSUM") as qps,
                tc.tile_pool(name="denp", bufs=1, space="PSUM") as denp,
                tc.tile_pool(name="ndp", bufs=2, space="PSUM") as ndp,
                tc.tile_pool(name="bcp", bufs=1, space="PSUM") as bcp,
                tc.tile_pool(name="yps", bufs=2, space="PSUM") as yps,
                tc.tile_pool(name="qfm", bufs=1) as qfmp,
                tc.tile_pool(name="att", bufs=1) as attp,
                tc.tile_pool(name="rec", bufs=2) as recp,
                tc.tile_pool(name="elu2", bufs=2) as elu2,
                tc.tile_pool(name="ysb", bufs=2) as ysbp,
            ):
                owT_sb = w2.tile([128, EC, E], BF16, tag="owT")
                nc.sync.dma_start(
                    out=owT_sb[:], in_=owT.rearrange("(c p) e -> p c e",
                                                     p=128))

                kv_red = kvx.tile([128, HP, D + 1], F32, tag="kvred")
                nc.sync.dma_start(out=kv_red[:], in_=cc_out[:])
                kv_ext = kvx.tile([128, HP, D + 1], BF16, tag="kvext")
                nc.vector.tensor_copy(kv_ext[:], kv_red[:])
                if debug_dump:
                    nc.sync.dma_start(out=dbg["kvred"][:], in_=kv_red[:])

                # KSmat[p, ec, h] = k_sum[h, p - 64*(h%2)] for h//2==ec else 0
                KSmat = kvx.tile([128, EC, 16], BF16, tag="ksmat")
                nc.vector.memset(KSmat[:], 0.0)
                for h in range(H):
                    base = 64 * (h % 2)
                    nc.vector.tensor_copy(
                        KSmat[base:base + 64, h // 2, h:h + 1],
                        kv_ext[base:base + 64, h // 2, D:D + 1])

                for c in range(NCH):
                    # q up-projection (feature-major) + bias + elu+1
                    qfm = qfmp.tile([128, EC, CH], BF16, tag="qfm")
                    for fc in range(EC):
                        ps = qps.tile([128, CH], F32, tag="qps")
                        for rc in range(RC):
                            nc.tensor.matmul(
                                ps[:], quT_sb[:, rc, fc * 128:(fc + 1) * 128],
                                qd_all[:, rc, bass.ds(c * CH, CH)],
                                start=(rc == 0), stop=False)
                        nc.tensor.matmul(
                            ps[:], qu_brow[:, fc * 128:(fc + 1) * 128],
                            ones_row[:, 0:CH], start=False, stop=True)
                        m = elu2.tile([128, CH], F32, tag="m2")
                        nc.vector.tensor_scalar_min(m[:], ps[:], 0.0)
                        e = elu2.tile([128, CH], F32, tag="e2")
                        nc.scalar.activation(e[:], m[:], Exp)
                        r = elu2.tile([128, CH], F32, tag="r2")
                        nc.scalar.activation(r[:], ps[:], Relu)
                        nc.vector.tensor_add(qfm[:, fc, :], e[:], r[:])

                    # denominators for all 16 heads at once
                    den = denp.tile([16, CH], F32, tag="den")
                    for ec in range(EC):
                        nc.tensor.matmul(den[:], KSmat[:, ec, :],
                                         qfm[:, ec, :],
                                         start=(ec == 0), stop=(ec == EC - 1))
                    rec = recp.tile([16, CH], F32, tag="rec")
                    nc.vector.reciprocal_approx_fast(rec[:], den[:])
                    recb = recp.tile([16, CH], BF16, tag="recb")
                    nc.vector.tensor_copy(recb[:], rec[:])
                    if debug_dump and c == 0:
                        dent = recp.tile([16, CH], F32, tag="dent")
                        nc.vector.tensor_copy(dent[:], den[:])
                        nc.sync.dma_start(out=dbg["den"][:], in_=dent[:])
                        nc.sync.dma_start(out=dbg["rec"][:], in_=rec[:])
                        nc.gpsimd.dma_start(out=dbg["qfm"][:], in_=qfm[:])

                    # attention per head pair: packed num matmuls + bcast mul
                    att = attp.tile([128, EC, CH], BF16, tag="att")
                    for hp in range(HP):
                        nd = ndp.tile([128, CH], F32, tag="nd")
                        nc.tensor.matmul(nd[0:64, :], kv_ext[0:64, hp, 0:D],
                                         qfm[0:64, hp, :],
                                         start=True, stop=True)
                        nc.tensor.matmul(nd[64:128, :],
                                         kv_ext[64:128, hp, 0:D],
                                         qfm[64:128, hp, :],
                                         start=True, stop=True,
                                         tile_position=(64, 64))
                        bc = bcp.tile([128, CH], F32, tag="bc")
                        nc.tensor.matmul(bc[:], ppat[:, hp, :], recb[:],
                                         start=True, stop=True)
                        bcs = elu2.tile([128, CH], F32, tag="bcs")
                        nc.scalar.activation(bcs[:], bc[:], Copy)
                        nc.vector.tensor_mul(att[:, hp, :], nd[:], bcs[:])
                        if debug_dump and c == 0 and hp == 0:
                            ndt = elu2.tile([128, CH], F32, tag="ndt")
                            nc.vector.tensor_copy(ndt[:], nd[:])
                            nc.sync.dma_start(out=dbg["nd0"][:], in_=ndt[:])
                            nc.sync.dma_start(out=dbg["bc0"][:], in_=bcs[:])

                    if debug_dump and c == 0:
                        nc.gpsimd.dma_start(out=dbg["att"][:], in_=att[:])

                    # output projection (token-major) + bias
                    for tb in range(TB):
                        ysb = ysbp.tile([128, E], F32, tag="ysb")
                        for fo in range(FC5):
                            py = yps.tile([128, 512], F32, tag="yps")
                            for ec in range(EC):
                                nc.tensor.matmul(
                                    py[:],
                                    att[:, ec, tb * 128:(tb + 1) * 128],
                                    owT_sb[:, ec, fo * 512:(fo + 1) * 512],
                                    start=(ec == 0), stop=False)
                            nc.tensor.matmul(
                                py[:], ones_row[:, 0:128],
                                ob_row[:, fo * 512:(fo + 1) * 512],
                                start=False, stop=True)
                            nc.scalar.activation(
                                ysb[:, fo * 512:(fo + 1) * 512], py[:], Copy)
                        r0 = c * CH + tb * 128
                        nc.sync.dma_start(out=y[r0:r0 + 128, :], in_=ysb[:])

    nc.compile()
    return nc


_NC_CACHE = {}


def _get_nc(T, n_cores, groups):
    key = (T, n_cores, tuple(tuple(g) for g in groups))
    if key not in _NC_CACHE:
        _NC_CACHE[key] = build_nc(T, n_cores, groups)
    return _NC_CACHE[key]


def _make_in_maps(inputs):
    bf = ml_dtypes.bfloat16
    query = np.asarray(inputs["query"], dtype=np.float32)
    key = np.asarray(inputs["key"], dtype=np.float32)
    value = np.asarray(inputs["value"], dtype=np.float32)

    weights = {
        "qdT": np.asarray(inputs["qd_w"], np.float32).T.astype(bf),
        "kdT": np.asarray(inputs["kd_w"], np.float32).T.astype(bf),
        "vdT": np.asarray(inputs["vd_w"], np.float32).T.astype(bf),
        "quT": np.asarray(inputs["qu_w"], np.float32).T.astype(bf),
        "kuT": np.asarray(inputs["ku_w"], np.float32).T.astype(bf),
        "vuT": np.asarray(inputs["vu_w"], np.float32).T.astype(bf),
        "owT": np.asarray(inputs["out_w"], np.float32).T.astype(bf),
        "qu_b": np.asarray(inputs["qu_b"], np.float32).astype(bf),
        "ku_b": np.asarray(inputs["ku_b"], np.float32).astype(bf),
        "vu_b": np.asarray(inputs["vu_b"], np.float32).astype(bf),
        "out_b": np.asarray(inputs["out_b"], np.float32).astype(bf),
    }
    HP = H // 2
    ppat = np.zeros((16, HP * 128), dtype=np.float32)
    for hp in range(HP):
        ppat[2 * hp, hp * 128:hp * 128 + 64] = 1.0
        ppat[2 * hp + 1, hp * 128 + 64:hp * 128 + 128] = 1.0
    weights["ppat"] = ppat.astype(bf)

    half = S // 2
    in_maps = []
    for c in range(N_CORES):
        bi, hi = c // 2, c % 2
        sl = slice(hi * half, (hi + 1) * half)
        m = {
            "xq": query[bi, sl].T.astype(bf),
            "xk": key[bi, sl].T.astype(bf),
            "xv": value[bi, sl].T.astype(bf),
        }
        m.update(weights)
        in_maps.append(m)
    return in_maps


def kernel(**inputs):
    b, s, e = np.asarray(inputs["query"]).shape
    assert (b, s, e) == (B, S, E)

    T = B * S // N_CORES  # 2048 tokens per core
    half = S // 2
    groups = [[0, 1], [2, 3], [4, 5], [6, 7]]
    nc = _get_nc(T, N_CORES, groups)

    in_maps = _make_in_maps(inputs)
    res = run_bass_kernel_spmd(nc, in_maps, list(range(N_CORES)))

    out = np.empty((B, S, E), dtype=np.float32)
    for c in range(N_CORES):
        bi, hi = c // 2, c % 2
        out[bi, hi * half:(hi + 1) * half] = res.results[c]["y"]
    return out


# revision 31
# speedup vs baseline: 1.0449x; 1.0449x over previous
"""Trainium2 Bass kernel for nn_CustomMultiheadAttention (linear attention with
low-rank QKV projections), SPMD over 8 NeuronCores.

Sharding: (batch, seq-half) -> core.  Core c handles batch c//2, sequence half
c%2 (2048 tokens).  Pairwise AllReduce of kv/k_sum between the two cores
sharing a batch.

Host-side prep: x and all weights are pre-transposed and cast to bf16 in
numpy so the device sees feature-major operands directly (no PE transposes,
no cast copies).  Bias adds are folded into the matmul accumulation groups as
K=1 ones-vector matmuls.  Attention denominators are accumulated for all 16
heads at once via a [128,16] selection-matrix matmul, inverted with one
reciprocal_approx_fast per chunk, and broadcast back with a K=16 pattern
matmul; head pairs share one [128,CH] PSUM tile so the divide is a single
tensor_tensor per pair.

elu(z)+1 is computed as exp(min(z,0)) + relu(z)  (exact).
"""

import numpy as np
import ml_dtypes

import concourse.bass as bass
import concourse.tile as tile
from concourse import bacc, mybir
from concourse.bass_utils import run_bass_kernel_spmd

F32 = mybir.dt.float32
BF16 = mybir.dt.bfloat16

B, S, E, H, R = 4, 4096, 1024, 16, 512
D = E // H  # 64
N_CORES = 8


def build_nc(T, n_cores, groups, debug_dump=False):
    """Build the SPMD bass kernel for T tokens per core."""
    CH = min(512, T)        # tokens per chunk
    NCH = T // CH           # chunks
    TB = CH // 128          # 128-token blocks per chunk
    EC = E // 128           # 8 feature chunks of 128
    RC = R // 128           # 4
    FC5 = E // 512          # 2 (512-wide f chunks)
    HP = H // 2             # head pairs
    hpf = 512 // D          # heads per 512-wide chunk = 8

    nc = bacc.Bacc("TRN2", target_bir_lowering=False, debug=False,
                   num_devices=n_cores)

    xq = nc.declare_dram_parameter("xq", [E, T], BF16, isOutput=False).ap()
    xk = nc.declare_dram_parameter("xk", [E, T], BF16, isOutput=False).ap()
    xv = nc.declare_dram_parameter("xv", [E, T], BF16, isOutput=False).ap()
    # host passes W.T for every weight
    qdT = nc.declare_dram_parameter("qdT", [E, R], BF16, isOutput=False).ap()
    kdT = nc.declare_dram_parameter("kdT", [E, R], BF16, isOutput=False).ap()
    vdT = nc.declare_dram_parameter("vdT", [E, R], BF16, isOutput=False).ap()
    quT = nc.declare_dram_parameter("quT", [R, E], BF16, isOutput=False).ap()
    kuT = nc.declare_dram_parameter("kuT", [R, E], BF16, isOutput=False).ap()
    vuT = nc.declare_dram_parameter("vuT", [R, E], BF16, isOutput=False).ap()
    owT = nc.declare_dram_parameter("owT", [E, E], BF16, isOutput=False).ap()
    qu_b = nc.declare_dram_parameter("qu_b", [E], F32, isOutput=False).ap()
    ku_b = nc.declare_dram_parameter("ku_b", [E], BF16, isOutput=False).ap()
    vu_b = nc.declare_dram_parameter("vu_b", [E], BF16, isOutput=False).ap()
    out_b = nc.declare_dram_parameter("out_b", [E], BF16, isOutput=False).ap()
    # host-built broadcast pattern: ppat[h, hp*128+p] = 1 iff head h covers
    # partition p of pair hp (p<64 -> even head, p>=64 -> odd head)
    ppat_d = nc.declare_dram_parameter("ppat", [16, HP * 128], BF16,
                                       isOutput=False).ap()
    y = nc.declare_dram_parameter("y", [T, E], F32, isOutput=True).ap()

    # kv | k_sum buffers: head pair hp -> partitions [0:64] = head 2*hp,
    # [64:128] = head 2*hp+1
    cc_in = nc.dram_tensor("cc_in", [128, HP, D + 1], F32).ap()
    cc_out = nc.dram_tensor("cc_out", [128, HP, D + 1], F32).ap()

    dbg = {}
    if debug_dump:
        for nm, shp in (("qd", [128, RC, T]), ("ktm", [128, TB, H, D + 1]),
                        ("vtm", [128, TB, H, D + 1]), ("kvacc", [128, HP, D + 1]),
                        ("kvred", [128, HP, D + 1]), ("qfm", [128, EC, CH]),
                        ("den", [16, CH]), ("rec", [16, CH]),
                        ("nd0", [128, CH]), ("bc0", [128, CH]),
                        ("att", [128, EC, CH])):
            dbg[nm] = nc.declare_dram_parameter(f"dbg_{nm}", shp, F32,
                                                isOutput=True).ap()

    Exp = mybir.ActivationFunctionType.Exp
    Relu = mybir.ActivationFunctionType.Relu
    Copy = mybir.ActivationFunctionType.Copy

    with tile.TileContext(nc) as tc:
        with (
            tc.tile_pool(name="const", bufs=1) as const,
            tc.tile_pool(name="persist", bufs=1) as persist,
            tc.tile_pool(name="w2a", bufs=1) as w2a,
        ):
            ones_row = const.tile([1, 512], BF16)
            nc.vector.memset(ones_row[:], 1.0)

            # broadcast pattern: bc[p,t] = sum_h ppat[h,hp,p] * rec[h,t]
            ppat = const.tile([16, HP, 128], BF16)
            nc.sync.dma_start(
                out=ppat[:], in_=ppat_d.rearrange("h (c p) -> h c p", p=128))

            ku_brow = const.tile([1, E], BF16, tag="kub")
            vu_brow = const.tile([1, E], BF16, tag="vub")
            ob_row = const.tile([1, E], BF16, tag="ob")
            for t_, a_ in ((ku_brow, ku_b), (vu_brow, vu_b), (ob_row, out_b)):
                nc.sync.dma_start(out=t_[:],
                                  in_=a_.rearrange("(o f) -> o f", o=1))
            # qu_b as per-partition columns (feature-major bias)
            qu_bc = const.tile([128, EC], F32, tag="qubc")
            nc.sync.dma_start(out=qu_bc[:],
                              in_=qu_b.rearrange("(c p) -> p c", p=128))
            # vu_b / out_b broadcast along partitions (token-major adds)
            vu_bcast = const.tile([128, E], BF16, tag="vubc")
            nc.gpsimd.partition_broadcast(vu_bcast[:], vu_brow[:])
            ou_bcast = const.tile([128, E], BF16, tag="oubc")
            nc.gpsimd.partition_broadcast(ou_bcast[:], ob_row[:])

            qd_all = persist.tile([128, RC, T], BF16)     # Qd feature-major
            kv_acc = persist.tile([128, HP, D + 1], F32)  # kv | k_sum

            # ---------------- Phase 1 ----------------
            with (
                tc.tile_pool(name="wkv", bufs=1) as wkv,
            ):
                kdT_sb = wkv.tile([128, EC, R], BF16, tag="kdT")
                vdT_sb = wkv.tile([128, EC, R], BF16, tag="vdT")
                kuT_sb = wkv.tile([128, RC, E], BF16, tag="kuT")
                vuT_sb = wkv.tile([128, RC, E], BF16, tag="vuT")
                quT_sb = w2a.tile([128, RC, E], BF16, tag="quT")
                nc.sync.dma_start(
                    out=kdT_sb[:], in_=kdT.rearrange("(c p) r -> p c r", p=128))
                nc.sync.dma_start(
                    out=vdT_sb[:], in_=vdT.rearrange("(c p) r -> p c r", p=128))
                nc.sync.dma_start(
                    out=kuT_sb[:], in_=kuT.rearrange("(c p) e -> p c e", p=128))
                nc.sync.dma_start(
                    out=vuT_sb[:], in_=vuT.rearrange("(c p) e -> p c e", p=128))
                nc.sync.dma_start(
                    out=quT_sb[:], in_=quT.rearrange("(c p) e -> p c e", p=128))

                with (
                    tc.tile_pool(name="xfm", bufs=2) as xfmp,
                    tc.tile_pool(name="dpsum", bufs=2, space="PSUM") as dpsum,
                ):
                    def load_x_fm(x_ap, c):
                        xfm = xfmp.tile([128, EC, CH], BF16, tag="xfm")
                        nc.sync.dma_start(
                            out=xfm[:],
                            in_=x_ap.rearrange("(f p) t -> p f t", p=128)
                                    [:, :, c * CH:(c + 1) * CH])
                        return xfm

                    def down_proj(xfm, dwT, dst, dst_sl):
                        # dst[:, rb, dst_sl] = (W_down @ x^T) feature-major
                        for rb in range(RC):
                            ps = dpsum.tile([128, CH], F32, tag="dps")
                            for ec in range(EC):
                                nc.tensor.matmul(
                                    ps[:], dwT[:, ec, rb * 128:(rb + 1) * 128],
                                    xfm[:, ec, :],
                                    start=(ec == 0), stop=(ec == EC - 1))
                            nc.vector.tensor_copy(dst[:, rb, dst_sl], ps[:])

                    # --- 1a: q down-projection ---
                    with tc.tile_pool(name="w1q", bufs=1) as w1q:
                        qdT_sb = w1q.tile([128, EC, R], BF16, tag="qdT")
                        nc.sync.dma_start(
                            out=qdT_sb[:],
                            in_=qdT.rearrange("(c p) r -> p c r", p=128))
                        for c in range(NCH):
                            xfm = load_x_fm(xq, c)
                            down_proj(xfm, qdT_sb, qd_all,
                                      bass.ds(c * CH, CH))
                        if debug_dump:
                            nc.gpsimd.dma_start(out=dbg["qd"][:],
                                                in_=qd_all[:])

                    # --- 1b: k/v down+up, elu, kv accumulation ---
                    with (
                        tc.tile_pool(name="dd", bufs=2) as ddp,
                        tc.tile_pool(name="upsum", bufs=2,
                                     space="PSUM") as upsum,
                        tc.tile_pool(name="ktm", bufs=1) as ktmp,
                        tc.tile_pool(name="vtm", bufs=1) as vtmp,
                        tc.tile_pool(name="elu1", bufs=2) as elu1,
                        tc.tile_pool(name="kvps", bufs=4,
                                     space="PSUM") as kvps,
                    ):

                        def up_k(dd, ktm):
                            for tb in range(TB):
                                for fc in range(FC5):
                                    ps = upsum.tile([128, 512], F32,
                                                    tag="ups")
                                    for rc in range(RC):
                                        nc.tensor.matmul(
                                            ps[:],
                                            dd[:, rc,
                                               tb * 128:(tb + 1) * 128],
                                            kuT_sb[:, rc,
                                                   fc * 512:(fc + 1) * 512],
                                            start=(rc == 0), stop=False)
                                    nc.tensor.matmul(
                                        ps[:], ones_row[:, 0:128],
                                        ku_brow[:, fc * 512:(fc + 1) * 512],
                                        start=False, stop=True)
                                    m = elu1.tile([128, 512], F32, tag="m")
                                    nc.vector.tensor_scalar_min(
                                        m[:], ps[:], 0.0)
                                    e = elu1.tile([128, 512], F32, tag="e")
                                    nc.scalar.activation(e[:], m[:], Exp)
                                    r = elu1.tile([128, 512], F32, tag="r")
                                    nc.scalar.activation(r[:], ps[:], Relu)
                                    dsl = ktm[:, tb,
                                              fc * hpf:(fc + 1) * hpf, 0:D]
                                    nc.vector.tensor_add(
                                        dsl,
                                        e[:].rearrange("p (h f) -> p h f",
                                                       h=hpf),
                                        r[:].rearrange("p (h f) -> p h f",
                                                       h=hpf))

                        def up_v(dd, vtm):
                            for tb in range(TB):
                                for fc in range(FC5):
                                    ps = upsum.tile([128, 512], F32,
                                                    tag="ups")
                                    for rc in range(RC):
                                        nc.tensor.matmul(
                                            ps[:],
                                            dd[:, rc,
                                               tb * 128:(tb + 1) * 128],
                                            vuT_sb[:, rc,
                                                   fc * 512:(fc + 1) * 512],
                                            start=(rc == 0),
                                            stop=(rc == RC - 1))
                                    dsl = vtm[:, tb,
                                              fc * hpf:(fc + 1) * hpf, 0:D]
                                    bsl = vu_bcast[:, fc * 512:(fc + 1) * 512]
                                    nc.vector.tensor_add(
                                        dsl,
                                        ps[:].rearrange("p (h f) -> p h f",
                                                        h=hpf),
                                        bsl.rearrange("p (h f) -> p h f",
                                                      h=hpf))

                        for c in range(NCH):
                            xfm = load_x_fm(xk, c)
                            ddk = ddp.tile([128, RC, CH], BF16, tag="dd")
                            down_proj(xfm, kdT_sb, ddk, slice(None))
                            ktm = ktmp.tile([128, TB, H, D + 1], BF16,
                                            tag="ktm")
                            up_k(ddk, ktm)

                            xfm = load_x_fm(xv, c)
                            ddv = ddp.tile([128, RC, CH], BF16, tag="dd")
                            down_proj(xfm, vdT_sb, ddv, slice(None))
                            vtm = vtmp.tile([128, TB, H, D + 1], BF16,
                                            tag="vtm")
                            nc.vector.memset(vtm[:, :, :, D:D + 1], 1.0)
                            up_v(ddv, vtm)

                            if debug_dump and c == 0:
                                nc.gpsimd.dma_start(out=dbg["ktm"][:],
                                                    in_=ktm[:])
                                nc.gpsimd.dma_start(out=dbg["vtm"][:],
                                                    in_=vtm[:])

                            for hp in range(HP):
                                pkv = kvps.tile([128, D + 1], F32, tag="kvps")
                                for tb in range(TB):
                                    nc.tensor.matmul(
                                        pkv[0:64, :],
                                        ktm[:, tb, 2 * hp, 0:D],
                                        vtm[:, tb, 2 * hp, 0:D + 1],
                                        start=(tb == 0), stop=(tb == TB - 1))
                                for tb in range(TB):
                                    nc.tensor.matmul(
                                        pkv[64:128, :],
                                        ktm[:, tb, 2 * hp + 1, 0:D],
                                        vtm[:, tb, 2 * hp + 1, 0:D + 1],
                                        start=(tb == 0), stop=(tb == TB - 1),
                                        tile_position=(0, 64))
                                acc_sl = kv_acc[:, hp, :]
                                if c == 0:
                                    nc.vector.tensor_copy(acc_sl, pkv[:])
                                else:
                                    nc.vector.tensor_add(acc_sl, acc_sl,
                                                         pkv[:])
                        if debug_dump:
                            nc.sync.dma_start(out=dbg["kvacc"][:],
                                              in_=kv_acc[:])

            # ---------------- AllReduce kv across the batch pair ----------
            nc.sync.dma_start(out=cc_in[:], in_=kv_acc[:])
            nc.gpsimd.collective_compute(
                "AllReduce", mybir.AluOpType.add,
                ins=[cc_in[:]], outs=[cc_out[:]],
                replica_groups=groups)

            # ---------------- Phase 2 ----------------
            with (
                tc.tile_pool(name="w2", bufs=1) as w2,
                tc.tile_pool(name="kvx", bufs=1) as kvx,
                tc.tile_pool(name="qps", bufs=2, space="PSUM") as qps,
                tc.tile_pool(name="denp", bufs=1, space="PSUM") as denp,
                tc.tile_pool(name="ndp", bufs=2, space="PSUM") as ndp,
                tc.tile_pool(name="bcp", bufs=1, space="PSUM") as bcp,
                tc.tile_pool(name="yps", bufs=2, space="PSUM") as yps,
                tc.tile_pool(name="qfm", bufs=1) as qfmp,
                tc.tile_pool(name="att", bufs=1) as attp,
                tc.tile_pool(name="rec", bufs=2) as recp,
                tc.tile_pool(name="elu2", bufs=2) as elu2,
                tc.tile_pool(name="ysb", bufs=2) as ysbp,
            ):
                owT_sb = w2.tile([128, EC, E], BF16, tag="owT")
                nc.sync.dma_start(
                    out=owT_sb[:], in_=owT.rearrange("(c p) e -> p c e",
                                                     p=128))

                kv_red = kvx.tile([128, HP, D + 1], F32, tag="kvred")
                nc.sync.dma_start(out=kv_red[:], in_=cc_out[:])
                kv_ext = kvx.tile([128, HP, D + 1], BF16, tag="kvext")
                nc.vector.tensor_copy(kv_ext[:], kv_red[:])
                if debug_dump:
                    nc.sync.dma_start(out=dbg["kvred"][:], in_=kv_red[:])

                # KSmat[p, ec, h] = k_sum[h, p - 64*(h%2)] for h//2==ec else 0
                KSmat = kvx.tile([128, EC, 16], BF16, tag="ksmat")
                nc.vector.memset(KSmat[:], 0.0)
                for h in range(H):
                    base = 64 * (h % 2)
                    nc.vector.tensor_copy(
                        KSmat[base:base + 64, h // 2, h:h + 1],
                        kv_ext[base:base + 64, h // 2, D:D + 1])

                for c in range(NCH):
                    # q up-projection (feature-major) + bias + elu+1
                    qfm = qfmp.tile([128, EC, CH], BF16, tag="qfm")
                    for fc in range(EC):
                        ps = qps.tile([128, CH], F32, tag="qps")
                        for rc in range(RC):
                            nc.tensor.matmul(
                                ps[:], quT_sb[:, rc, fc * 128:(fc + 1) * 128],
                                qd_all[:, rc, bass.ds(c * CH, CH)],
                                start=(rc == 0), stop=(rc == RC - 1))
                        m = elu2.tile([128, CH], F32, tag="m2")
                        nc.vector.tensor_scalar(
                            m[:], ps[:], qu_bc[:, fc:fc + 1], 0.0,
                            op0=mybir.AluOpType.add, op1=mybir.AluOpType.min)
                        e = elu2.tile([128, CH], F32, tag="e2")
                        nc.scalar.activation(e[:], m[:], Exp)
                        r = elu2.tile([128, CH], F32, tag="r2")
                        nc.scalar.activation(r[:], ps[:], Relu,
                                             bias=qu_bc[:, fc:fc + 1])
                        nc.vector.tensor_add(qfm[:, fc, :], e[:], r[:])

                    # denominators for all 16 heads at once
                    den = denp.tile([16, CH], F32, tag="den")
                    for ec in range(EC):
                        nc.tensor.matmul(den[:], KSmat[:, ec, :],
                                         qfm[:, ec, :],
                                         start=(ec == 0), stop=(ec == EC - 1))
                    rec = recp.tile([16, CH], F32, tag="rec")
                    nc.vector.reciprocal_approx_fast(rec[:], den[:])
                    recb = recp.tile([16, CH], BF16, tag="recb")
                    nc.vector.tensor_copy(recb[:], rec[:])
                    if debug_dump and c == 0:
                        dent = recp.tile([16, CH], F32, tag="dent")
                        nc.vector.tensor_copy(dent[:], den[:])
                        nc.sync.dma_start(out=dbg["den"][:], in_=dent[:])
                        nc.sync.dma_start(out=dbg["rec"][:], in_=rec[:])
                        nc.gpsimd.dma_start(out=dbg["qfm"][:], in_=qfm[:])

                    # attention per head pair: packed num matmuls + bcast mul
                    att = attp.tile([128, EC, CH], BF16, tag="att")
                    for hp in range(HP):
                        nd = ndp.tile([128, CH], F32, tag="nd")
                        nc.tensor.matmul(nd[0:64, :], kv_ext[0:64, hp, 0:D],
                                         qfm[0:64, hp, :],
                                         start=True, stop=True)
                        nc.tensor.matmul(nd[64:128, :],
                                         kv_ext[64:128, hp, 0:D],
                                         qfm[64:128, hp, :],
                                         start=True, stop=True,
                                         tile_position=(64, 64))
                        bc = bcp.tile([128, CH], F32, tag="bc")
                        nc.tensor.matmul(bc[:], ppat[:, hp, :], recb[:],
                                         start=True, stop=True)
                        bcs = elu2.tile([128, CH], F32, tag="bcs")
                        nc.scalar.activation(bcs[:], bc[:], Copy)
                        nc.vector.tensor_mul(att[:, hp, :], nd[:], bcs[:])
                        if debug_dump and c == 0 and hp == 0:
                            ndt = elu2.tile([128, CH], F32, tag="ndt")
                            nc.vector.tensor_copy(ndt[:], nd[:])
                            nc.sync.dma_start(out=dbg["nd0"][:], in_=ndt[:])
                            nc.sync.dma_start(out=dbg["bc0"][:], in_=bcs[:])

                    if debug_dump and c == 0:
                        nc.gpsimd.dma_start(out=dbg["att"][:], in_=att[:])

                    # output projection (token-major) + bias
                    ysb = ysbp.tile([128, TB, E], F32, tag="ysb")
                    for tb in range(TB):
                        for fo in range(FC5):
                            py = yps.tile([128, 512], F32, tag="yps")
                            for ec in range(EC):
                                nc.tensor.matmul(
                                    py[:],
                                    att[:, ec, tb * 128:(tb + 1) * 128],
                                    owT_sb[:, ec, fo * 512:(fo + 1) * 512],
                                    start=(ec == 0), stop=(ec == EC - 1))
                            nc.vector.tensor_add(
                                ysb[:, tb, fo * 512:(fo + 1) * 512], py[:],
                                ou_bcast[:, fo * 512:(fo + 1) * 512])
                    nc.sync.dma_start(
                        out=y.rearrange("(cc tb p) e -> p cc tb e",
                                        p=128, tb=TB)[:, c, :, :],
                        in_=ysb[:])

    nc.compile()
    return nc


_NC_CACHE = {}


def _get_nc(T, n_cores, groups):
    key = (T, n_cores, tuple(tuple(g) for g in groups))
    if key not in _NC_CACHE:
        _NC_CACHE[key] = build_nc(T, n_cores, groups)
    return _NC_CACHE[key]


def _make_in_maps(inputs):
    bf = ml_dtypes.bfloat16
    query = np.asarray(inputs["query"], dtype=np.float32)
    key = np.asarray(inputs["key"], dtype=np.float32)
    value = np.asarray(inputs["value"], dtype=np.float32)

    weights = {
        "qdT": np.asarray(inputs["qd_w"], np.float32).T.astype(bf),
        "kdT": np.asarray(inputs["kd_w"], np.float32).T.astype(bf),
        "vdT": np.asarray(inputs["vd_w"], np.float32).T.astype(bf),
        "quT": np.asarray(inputs["qu_w"], np.float32).T.astype(bf),
        "kuT": np.asarray(inputs["ku_w"], np.float32).T.astype(bf),
        "vuT": np.asarray(inputs["vu_w"], np.float32).T.astype(bf),
        "owT": np.asarray(inputs["out_w"], np.float32).T.astype(bf),
        "qu_b": np.asarray(inputs["qu_b"], np.float32),
        "ku_b": np.asarray(inputs["ku_b"], np.float32).astype(bf),
        "vu_b": np.asarray(inputs["vu_b"], np.float32).astype(bf),
        "out_b": np.asarray(inputs["out_b"], np.float32).astype(bf),
    }
    HP = H // 2
    ppat = np.zeros((16, HP * 128), dtype=np.float32)
    for hp in range(HP):
        ppat[2 * hp, hp * 128:hp * 128 + 64] = 1.0
        ppat[2 * hp + 1, hp * 128 + 64:hp * 128 + 128] = 1.0
    weights["ppat"] = ppat.astype(bf)

    half = S // 2
    in_maps = []
    for c in range(N_CORES):
        bi, hi = c // 2, c % 2
        sl = slice(hi * half, (hi + 1) * half)
        m = {
            "xq": query[bi, sl].T.astype(bf),
            "xk": key[bi, sl].T.astype(bf),
            "xv": value[bi, sl].T.astype(bf),
        }
        m.update(weights)
        in_maps.append(m)
    return in_maps


def kernel(**inputs):
    b, s, e = np.asarray(inputs["query"]).shape
    assert (b, s, e) == (B, S, E)

    T = B * S // N_CORES  # 2048 tokens per core
    half = S // 2
    groups = [[0, 1], [2, 3], [4, 5], [6, 7]]
    nc = _get_nc(T, N_CORES, groups)

    in_maps = _make_in_maps(inputs)
    res = run_bass_kernel_spmd(nc, in_maps, list(range(N_CORES)))

    out = np.empty((B, S, E), dtype=np.float32)
    for c in range(N_CORES):
        bi, hi = c // 2, c % 2
        out[bi, hi * half:(hi + 1) * half] = res.results[c]["y"]
    return out


# revision 34
# speedup vs baseline: 1.1296x; 1.0810x over previous
"""Trainium2 Bass kernel for nn_CustomMultiheadAttention (linear attention with
low-rank QKV projections), SPMD over 8 NeuronCores.

Sharding: (batch, seq-half) -> core.  Core c handles batch c//2, sequence half
c%2 (2048 tokens).  Pairwise AllReduce of kv/k_sum between the two cores
sharing a batch.

Host-side prep: x and all weights are pre-transposed and cast to bf16 in
numpy so the device sees feature-major operands directly (no PE transposes,
no cast copies).  Bias adds are folded into the matmul accumulation groups as
K=1 ones-vector matmuls.  Attention denominators are accumulated for all 16
heads at once via a [128,16] selection-matrix matmul, inverted with one
reciprocal_approx_fast per chunk, and broadcast back with a K=16 pattern
matmul; head pairs share one [128,CH] PSUM tile so the divide is a single
tensor_tensor per pair.

elu(z)+1 is computed as exp(min(z,0)) + relu(z)  (exact).
"""

import numpy as np
import ml_dtypes

import concourse.bass as bass
import concourse.tile as tile
from concourse import bacc, mybir
from concourse.bass_utils import run_bass_kernel_spmd

F32 = mybir.dt.float32
BF16 = mybir.dt.bfloat16

B, S, E, H, R = 4, 4096, 1024, 16, 512
D = E // H  # 64
N_CORES = 8


def build_nc(T, n_cores, groups, debug_dump=False):
    """Build the SPMD bass kernel for T tokens per core."""
    CH = min(512, T)        # tokens per chunk
    NCH = T // CH           # chunks
    TB = CH // 128          # 128-token blocks per chunk
    EC = E // 128           # 8 feature chunks of 128
    RC = R // 128           # 4
    FC5 = E // 512          # 2 (512-wide f chunks)
    HP = H // 2             # head pairs
    hpf = 512 // D          # heads per 512-wide chunk = 8

    nc = bacc.Bacc("TRN2", target_bir_lowering=False, debug=False,
                   num_devices=n_cores)

    xq = nc.declare_dram_parameter("xq", [E, T], BF16, isOutput=False).ap()
    xk = nc.declare_dram_parameter("xk", [E, T], BF16, isOutput=False).ap()
    xv = nc.declare_dram_parameter("xv", [E, T], BF16, isOutput=False).ap()
    # host passes W.T for every weight
    qdT = nc.declare_dram_parameter("qdT", [E, R], BF16, isOutput=False).ap()
    kdT = nc.declare_dram_parameter("kdT", [E, R], BF16, isOutput=False).ap()
    vdT = nc.declare_dram_parameter("vdT", [E, R], BF16, isOutput=False).ap()
    quT = nc.declare_dram_parameter("quT", [R, E], BF16, isOutput=False).ap()
    kuT = nc.declare_dram_parameter("kuT", [R, E], BF16, isOutput=False).ap()
    vuT = nc.declare_dram_parameter("vuT", [R, E], BF16, isOutput=False).ap()
    owT = nc.declare_dram_parameter("owT", [E, E], BF16, isOutput=False).ap()
    qu_b = nc.declare_dram_parameter("qu_b", [E], F32, isOutput=False).ap()
    ku_b = nc.declare_dram_parameter("ku_b", [E], BF16, isOutput=False).ap()
    vu_b = nc.declare_dram_parameter("vu_b", [E], BF16, isOutput=False).ap()
    out_b = nc.declare_dram_parameter("out_b", [E], BF16, isOutput=False).ap()
    # host-built broadcast pattern: ppat[h, hp*128+p] = 1 iff head h covers
    # partition p of pair hp (p<64 -> even head, p>=64 -> odd head)
    ppat_d = nc.declare_dram_parameter("ppat", [16, HP * 128], BF16,
                                       isOutput=False).ap()
    y = nc.declare_dram_parameter("y", [T, E], F32, isOutput=True).ap()

    # kv | k_sum buffers: head pair hp -> partitions [0:64] = head 2*hp,
    # [64:128] = head 2*hp+1
    cc_in = nc.dram_tensor("cc_in", [128, HP, D + 1], F32).ap()
    cc_out = nc.dram_tensor("cc_out", [128, HP, D + 1], F32).ap()

    dbg = {}
    if debug_dump:
        for nm, shp in (("qd", [128, RC, T]), ("ktm", [128, TB, H, D + 1]),
                        ("vtm", [128, TB, H, D + 1]), ("kvacc", [128, HP, D + 1]),
                        ("kvred", [128, HP, D + 1]), ("qfm", [128, EC, CH]),
                        ("den", [16, CH]), ("rec", [16, CH]),
                        ("nd0", [128, CH]), ("bc0", [128, CH]),
                        ("att", [128, EC, CH])):
            dbg[nm] = nc.declare_dram_parameter(f"dbg_{nm}", shp, F32,
                                                isOutput=True).ap()

    Exp = mybir.ActivationFunctionType.Exp
    Relu = mybir.ActivationFunctionType.Relu
    Copy = mybir.ActivationFunctionType.Copy

    with tile.TileContext(nc) as tc:
        with (
            tc.tile_pool(name="const", bufs=1) as const,
            tc.tile_pool(name="persist", bufs=1) as persist,
            tc.tile_pool(name="w2a", bufs=1) as w2a,
        ):
            ones_row = const.tile([1, 512], BF16)
            nc.vector.memset(ones_row[:], 1.0)

            # broadcast pattern: bc[p,t] = sum_h ppat[h,hp,p] * rec[h,t]
            ppat = const.tile([16, HP, 128], BF16)
            nc.sync.dma_start(
                out=ppat[:], in_=ppat_d.rearrange("h (c p) -> h c p", p=128))

            ku_brow = const.tile([1, E], BF16, tag="kub")
            vu_brow = const.tile([1, E], BF16, tag="vub")
            ob_row = const.tile([1, E], BF16, tag="ob")
            for t_, a_ in ((ku_brow, ku_b), (vu_brow, vu_b), (ob_row, out_b)):
                nc.sync.dma_start(out=t_[:],
                                  in_=a_.rearrange("(o f) -> o f", o=1))
            # qu_b as per-partition columns (feature-major bias)
            qu_bc = const.tile([128, EC], F32, tag="qubc")
            nc.sync.dma_start(out=qu_bc[:],
                              in_=qu_b.rearrange("(c p) -> p c", p=128))
            # vu_b / out_b broadcast along partitions (token-major adds)
            vu_bcast = const.tile([128, E], BF16, tag="vubc")
            nc.gpsimd.partition_broadcast(vu_bcast[:], vu_brow[:])
            ou_bcast = const.tile([128, E], BF16, tag="oubc")
            nc.gpsimd.partition_broadcast(ou_bcast[:], ob_row[:])

            qd_all = persist.tile([128, RC, T], BF16)     # Qd feature-major
            kv_acc = persist.tile([128, HP, D + 1], F32)  # kv | k_sum

            # ---------------- Phase 1 ----------------
            with (
                tc.tile_pool(name="wkv", bufs=1) as wkv,
            ):
                kdT_sb = wkv.tile([128, EC, R], BF16, tag="kdT")
                vdT_sb = wkv.tile([128, EC, R], BF16, tag="vdT")
                kuT_sb = wkv.tile([128, RC, E], BF16, tag="kuT")
                vuT_sb = wkv.tile([128, RC, E], BF16, tag="vuT")
                qdT_sb = wkv.tile([128, EC, R], BF16, tag="qdT")
                quT_sb = w2a.tile([128, RC, E], BF16, tag="quT")
                nc.sync.dma_start(
                    out=qdT_sb[:], in_=qdT.rearrange("(c p) r -> p c r", p=128))
                nc.sync.dma_start(
                    out=kdT_sb[:], in_=kdT.rearrange("(c p) r -> p c r", p=128))
                nc.sync.dma_start(
                    out=vdT_sb[:], in_=vdT.rearrange("(c p) r -> p c r", p=128))
                nc.sync.dma_start(
                    out=kuT_sb[:], in_=kuT.rearrange("(c p) e -> p c e", p=128))
                nc.sync.dma_start(
                    out=vuT_sb[:], in_=vuT.rearrange("(c p) e -> p c e", p=128))
                nc.sync.dma_start(
                    out=quT_sb[:], in_=quT.rearrange("(c p) e -> p c e", p=128))

                with (
                    tc.tile_pool(name="xfm", bufs=2) as xfmp,
                    tc.tile_pool(name="dpsum", bufs=2, space="PSUM") as dpsum,
                ):
                    def load_x_fm(x_ap, c):
                        xfm = xfmp.tile([128, EC, CH], BF16, tag="xfm")
                        nc.sync.dma_start(
                            out=xfm[:],
                            in_=x_ap.rearrange("(f p) t -> p f t", p=128)
                                    [:, :, c * CH:(c + 1) * CH])
                        return xfm

                    def down_proj(xfm, dwT, dst, dst_sl):
                        # dst[:, rb, dst_sl] = (W_down @ x^T) feature-major
                        for rb in range(RC):
                            ps = dpsum.tile([128, CH], F32, tag="dps")
                            for ec in range(EC):
                                nc.tensor.matmul(
                                    ps[:], dwT[:, ec, rb * 128:(rb + 1) * 128],
                                    xfm[:, ec, :],
                                    start=(ec == 0), stop=(ec == EC - 1))
                            nc.vector.tensor_copy(dst[:, rb, dst_sl], ps[:])

                    # --- 1a: k/v down+up, elu, kv accumulation (first, so
                    # the AllReduce overlaps the q down-projection) ---
                    with (
                        tc.tile_pool(name="dd", bufs=2) as ddp,
                        tc.tile_pool(name="upsum", bufs=2,
                                     space="PSUM") as upsum,
                        tc.tile_pool(name="ktm", bufs=1) as ktmp,
                        tc.tile_pool(name="vtm", bufs=1) as vtmp,
                        tc.tile_pool(name="elu1", bufs=2) as elu1,
                        tc.tile_pool(name="kvps", bufs=4,
                                     space="PSUM") as kvps,
                    ):

                        def up_k(dd, ktm):
                            for tb in range(TB):
                                for fc in range(FC5):
                                    ps = upsum.tile([128, 512], F32,
                                                    tag="ups")
                                    for rc in range(RC):
                                        nc.tensor.matmul(
                                            ps[:],
                                            dd[:, rc,
                                               tb * 128:(tb + 1) * 128],
                                            kuT_sb[:, rc,
                                                   fc * 512:(fc + 1) * 512],
                                            start=(rc == 0), stop=False)
                                    nc.tensor.matmul(
                                        ps[:], ones_row[:, 0:128],
                                        ku_brow[:, fc * 512:(fc + 1) * 512],
                                        start=False, stop=True)
                                    m = elu1.tile([128, 512], F32, tag="m")
                                    nc.vector.tensor_scalar_min(
                                        m[:], ps[:], 0.0)
                                    e = elu1.tile([128, 512], F32, tag="e")
                                    nc.scalar.activation(e[:], m[:], Exp)
                                    r = elu1.tile([128, 512], F32, tag="r")
                                    nc.scalar.activation(r[:], ps[:], Relu)
                                    dsl = ktm[:, tb,
                                              fc * hpf:(fc + 1) * hpf, 0:D]
                                    nc.vector.tensor_add(
                                        dsl,
                                        e[:].rearrange("p (h f) -> p h f",
                                                       h=hpf),
                                        r[:].rearrange("p (h f) -> p h f",
                                                       h=hpf))

                        def up_v(dd, vtm):
                            for tb in range(TB):
                                for fc in range(FC5):
                                    ps = upsum.tile([128, 512], F32,
                                                    tag="ups")
                                    for rc in range(RC):
                                        nc.tensor.matmul(
                                            ps[:],
                                            dd[:, rc,
                                               tb * 128:(tb + 1) * 128],
                                            vuT_sb[:, rc,
                                                   fc * 512:(fc + 1) * 512],
                                            start=(rc == 0),
                                            stop=(rc == RC - 1))
                                    dsl = vtm[:, tb,
                                              fc * hpf:(fc + 1) * hpf, 0:D]
                                    bsl = vu_bcast[:, fc * 512:(fc + 1) * 512]
                                    nc.vector.tensor_add(
                                        dsl,
                                        ps[:].rearrange("p (h f) -> p h f",
                                                        h=hpf),
                                        bsl.rearrange("p (h f) -> p h f",
                                                      h=hpf))

                        for c in range(NCH):
                            xfm = load_x_fm(xk, c)
                            ddk = ddp.tile([128, RC, CH], BF16, tag="dd")
                            down_proj(xfm, kdT_sb, ddk, slice(None))
                            ktm = ktmp.tile([128, TB, H, D + 1], BF16,
                                            tag="ktm")
                            up_k(ddk, ktm)

                            xfm = load_x_fm(xv, c)
                            ddv = ddp.tile([128, RC, CH], BF16, tag="dd")
                            down_proj(xfm, vdT_sb, ddv, slice(None))
                            vtm = vtmp.tile([128, TB, H, D + 1], BF16,
                                            tag="vtm")
                            nc.vector.memset(vtm[:, :, :, D:D + 1], 1.0)
                            up_v(ddv, vtm)

                            if debug_dump and c == 0:
                                nc.gpsimd.dma_start(out=dbg["ktm"][:],
                                                    in_=ktm[:])
                                nc.gpsimd.dma_start(out=dbg["vtm"][:],
                                                    in_=vtm[:])

                            for hp in range(HP):
                                pkv = kvps.tile([128, D + 1], F32, tag="kvps")
                                for tb in range(TB):
                                    nc.tensor.matmul(
                                        pkv[0:64, :],
                                        ktm[:, tb, 2 * hp, 0:D],
                                        vtm[:, tb, 2 * hp, 0:D + 1],
                                        start=(tb == 0), stop=(tb == TB - 1))
                                for tb in range(TB):
                                    nc.tensor.matmul(
                                        pkv[64:128, :],
                                        ktm[:, tb, 2 * hp + 1, 0:D],
                                        vtm[:, tb, 2 * hp + 1, 0:D + 1],
                                        start=(tb == 0), stop=(tb == TB - 1),
                                        tile_position=(0, 64))
                                acc_sl = kv_acc[:, hp, :]
                                if c == 0:
                                    nc.vector.tensor_copy(acc_sl, pkv[:])
                                else:
                                    nc.vector.tensor_add(acc_sl, acc_sl,
                                                         pkv[:])
                        if debug_dump:
                            nc.sync.dma_start(out=dbg["kvacc"][:],
                                              in_=kv_acc[:])

                    # ------- AllReduce kv across the batch pair (overlaps
                    # the q down-projection below) -------
                    nc.sync.dma_start(out=cc_in[:], in_=kv_acc[:])
                    nc.gpsimd.collective_compute(
                        "AllReduce", mybir.AluOpType.add,
                        ins=[cc_in[:]], outs=[cc_out[:]],
                        replica_groups=groups)

                    # --- 1b: q down-projection ---
                    for c in range(NCH):
                        xfm = load_x_fm(xq, c)
                        down_proj(xfm, qdT_sb, qd_all, bass.ds(c * CH, CH))
                    if debug_dump:
                        nc.gpsimd.dma_start(out=dbg["qd"][:], in_=qd_all[:])

            # ---------------- Phase 2 ----------------
            with (
                tc.tile_pool(name="w2", bufs=1) as w2,
                tc.tile_pool(name="kvx", bufs=1) as kvx,
                tc.tile_pool(name="qps", bufs=2, space="PSUM") as qps,
                tc.tile_pool(name="denp", bufs=1, space="PSUM") as denp,
                tc.tile_pool(name="ndp", bufs=2, space="PSUM") as ndp,
                tc.tile_pool(name="bcp", bufs=1, space="PSUM") as bcp,
                tc.tile_pool(name="yps", bufs=2, space="PSUM") as yps,
                tc.tile_pool(name="qfm", bufs=1) as qfmp,
                tc.tile_pool(name="att", bufs=1) as attp,
                tc.tile_pool(name="rec", bufs=2) as recp,
                tc.tile_pool(name="elu2", bufs=2) as elu2,
                tc.tile_pool(name="ysb", bufs=2) as ysbp,
            ):
                owT_sb = w2.tile([128, EC, E], BF16, tag="owT")
                nc.sync.dma_start(
                    out=owT_sb[:], in_=owT.rearrange("(c p) e -> p c e",
                                                     p=128))

                kv_red = kvx.tile([128, HP, D + 1], F32, tag="kvred")
                nc.sync.dma_start(out=kv_red[:], in_=cc_out[:])
                kv_ext = kvx.tile([128, HP, D + 1], BF16, tag="kvext")
                nc.vector.tensor_copy(kv_ext[:], kv_red[:])
                if debug_dump:
                    nc.sync.dma_start(out=dbg["kvred"][:], in_=kv_red[:])

                # KSmat[p, ec, h] = k_sum[h, p - 64*(h%2)] for h//2==ec else 0
                KSmat = kvx.tile([128, EC, 16], BF16, tag="ksmat")
                nc.vector.memset(KSmat[:], 0.0)
                for h in range(H):
                    base = 64 * (h % 2)
                    nc.vector.tensor_copy(
                        KSmat[base:base + 64, h // 2, h:h + 1],
                        kv_ext[base:base + 64, h // 2, D:D + 1])

                for c in range(NCH):
                    # q up-projection (feature-major) + bias + elu+1
                    qfm = qfmp.tile([128, EC, CH], BF16, tag="qfm")
                    for fc in range(EC):
                        ps = qps.tile([128, CH], F32, tag="qps")
                        for rc in range(RC):
                            nc.tensor.matmul(
                                ps[:], quT_sb[:, rc, fc * 128:(fc + 1) * 128],
                                qd_all[:, rc, bass.ds(c * CH, CH)],
                                start=(rc == 0), stop=(rc == RC - 1))
                        m = elu2.tile([128, CH], F32, tag="m2")
                        nc.vector.tensor_scalar(
                            m[:], ps[:], qu_bc[:, fc:fc + 1], 0.0,
                            op0=mybir.AluOpType.add, op1=mybir.AluOpType.min)
                        e = elu2.tile([128, CH], F32, tag="e2")
                        nc.scalar.activation(e[:], m[:], Exp)
                        r = elu2.tile([128, CH], F32, tag="r2")
                        nc.scalar.activation(r[:], ps[:], Relu,
                                             bias=qu_bc[:, fc:fc + 1])
                        nc.vector.tensor_add(qfm[:, fc, :], e[:], r[:])

                    # denominators for all 16 heads at once
                    den = denp.tile([16, CH], F32, tag="den")
                    for ec in range(EC):
                        nc.tensor.matmul(den[:], KSmat[:, ec, :],
                                         qfm[:, ec, :],
                                         start=(ec == 0), stop=(ec == EC - 1))
                    rec = recp.tile([16, CH], F32, tag="rec")
                    nc.vector.reciprocal_approx_fast(rec[:], den[:])
                    recb = recp.tile([16, CH], BF16, tag="recb")
                    nc.vector.tensor_copy(recb[:], rec[:])
                    if debug_dump and c == 0:
                        dent = recp.tile([16, CH], F32, tag="dent")
                        nc.vector.tensor_copy(dent[:], den[:])
                        nc.sync.dma_start(out=dbg["den"][:], in_=dent[:])
                        nc.sync.dma_start(out=dbg["rec"][:], in_=rec[:])
                        nc.gpsimd.dma_start(out=dbg["qfm"][:], in_=qfm[:])

                    # attention per head pair: packed num matmuls + bcast mul
                    att = attp.tile([128, EC, CH], BF16, tag="att")
                    for hp in range(HP):
                        nd = ndp.tile([128, CH], F32, tag="nd")
                        nc.tensor.matmul(nd[0:64, :], kv_ext[0:64, hp, 0:D],
                                         qfm[0:64, hp, :],
                                         start=True, stop=True)
                        nc.tensor.matmul(nd[64:128, :],
                                         kv_ext[64:128, hp, 0:D],
                                         qfm[64:128, hp, :],
                                         start=True, stop=True,
                                         tile_position=(64, 64))
                        bc = bcp.tile([128, CH], F32, tag="bc")
                        nc.tensor.matmul(bc[:], ppat[:, hp, :], recb[:],
                                         start=True, stop=True)
                        bcs = elu2.tile([128, CH], F32, tag="bcs")
                        nc.scalar.activation(bcs[:], bc[:], Copy)
                        nc.vector.tensor_mul(att[:, hp, :], nd[:], bcs[:])
                        if debug_dump and c == 0 and hp == 0:
                            ndt = elu2.tile([128, CH], F32, tag="ndt")
                            nc.vector.tensor_copy(ndt[:], nd[:])
                            nc.sync.dma_start(out=dbg["nd0"][:], in_=ndt[:])
                            nc.sync.dma_start(out=dbg["bc0"][:], in_=bcs[:])

                    if debug_dump and c == 0:
                        nc.gpsimd.dma_start(out=dbg["att"][:], in_=att[:])

                    # output projection (token-major) + bias
                    ysb = ysbp.tile([128, TB, E], F32, tag="ysb")
                    for tb in range(TB):
                        for fo in range(FC5):
                            py = yps.tile([128, 512], F32, tag="yps")
                            for ec in range(EC):
                                nc.tensor.matmul(
                                    py[:],
                                    att[:, ec, tb * 128:(tb + 1) * 128],
                                    owT_sb[:, ec, fo * 512:(fo + 1) * 512],
                                    start=(ec == 0), stop=(ec == EC - 1))
                            nc.vector.tensor_add(
                                ysb[:, tb, fo * 512:(fo + 1) * 512], py[:],
                                ou_bcast[:, fo * 512:(fo + 1) * 512])
                    nc.sync.dma_start(
                        out=y.rearrange("(cc tb p) e -> p cc tb e",
                                        p=128, tb=TB)[:, c, :, :],
                        in_=ysb[:])

    nc.compile()
    return nc


_NC_CACHE = {}


def _get_nc(T, n_cores, groups):
    key = (T, n_cores, tuple(tuple(g) for g in groups))
    if key not in _NC_CACHE:
        _NC_CACHE[key] = build_nc(T, n_cores, groups)
    return _NC_CACHE[key]


def _make_in_maps(inputs):
    bf = ml_dtypes.bfloat16
    query = np.asarray(inputs["query"], dtype=np.float32)
    key = np.asarray(inputs["key"], dtype=np.float32)
    value = np.asarray(inputs["value"], dtype=np.float32)

    weights = {
        "qdT": np.asarray(inputs["qd_w"], np.float32).T.astype(bf),
        "kdT": np.asarray(inputs["kd_w"], np.float32).T.astype(bf),
        "vdT": np.asarray(inputs["vd_w"], np.float32).T.astype(bf),
        "quT": np.asarray(inputs["qu_w"], np.float32).T.astype(bf),
        "kuT": np.asarray(inputs["ku_w"], np.float32).T.astype(bf),
        "vuT": np.asarray(inputs["vu_w"], np.float32).T.astype(bf),
        "owT": np.asarray(inputs["out_w"], np.float32).T.astype(bf),
        "qu_b": np.asarray(inputs["qu_b"], np.float32),
        "ku_b": np.asarray(inputs["ku_b"], np.float32).astype(bf),
        "vu_b": np.asarray(inputs["vu_b"], np.float32).astype(bf),
        "out_b": np.asarray(inputs["out_b"], np.float32).astype(bf),
    }
    HP = H // 2
    ppat = np.zeros((16, HP * 128), dtype=np.float32)
    for hp in range(HP):
        ppat[2 * hp, hp * 128:hp * 128 + 64] = 1.0
        ppat[2 * hp + 1, hp * 128 + 64:hp * 128 + 128] = 1.0
    weights["ppat"] = ppat.astype(bf)

    half = S // 2
    in_maps = []
    for c in range(N_CORES):
        bi, hi = c // 2, c % 2
        sl = slice(hi * half, (hi + 1) * half)
        m = {
            "xq": query[bi, sl].T.astype(bf),
            "xk": key[bi, sl].T.astype(bf),
            "xv": value[bi, sl].T.astype(bf),
        }
        m.update(weights)
        in_maps.append(m)
    return in_maps


def kernel(**inputs):
    b, s, e = np.asarray(inputs["query"]).shape
    assert (b, s, e) == (B, S, E)

    T = B * S // N_CORES  # 2048 tokens per core
    half = S // 2
    groups = [[0, 1], [2, 3], [4, 5], [6, 7]]
    nc = _get_nc(T, N_CORES, groups)

    in_maps = _make_in_maps(inputs)
    res = run_bass_kernel_spmd(nc, in_maps, list(range(N_CORES)))

    out = np.empty((B, S, E), dtype=np.float32)
    for c in range(N_CORES):
        bi, hi = c // 2, c % 2
        out[bi, hi * half:(hi + 1) * half] = res.results[c]["y"]
    return out


# revision 40
# speedup vs baseline: 1.1513x; 1.0192x over previous
"""Trainium2 Bass kernel for nn_CustomMultiheadAttention (linear attention with
low-rank QKV projections), SPMD over 8 NeuronCores.

Sharding: (batch, seq-half) -> core.  Core c handles batch c//2, sequence half
c%2 (2048 tokens).  Pairwise AllReduce of kv/k_sum between the two cores
sharing a batch.

Host-side prep: x and all weights are pre-transposed and cast to bf16 in
numpy so the device sees feature-major operands directly (no PE transposes,
no cast copies).  Bias adds are folded into the matmul accumulation groups as
K=1 ones-vector matmuls.  Attention denominators are accumulated for all 16
heads at once via a [128,16] selection-matrix matmul, inverted with one
reciprocal_approx_fast per chunk, and broadcast back with a K=16 pattern
matmul; head pairs share one [128,CH] PSUM tile so the divide is a single
tensor_tensor per pair.

elu(z)+1 is computed as exp(min(z,0)) + relu(z)  (exact).
"""

import numpy as np
import ml_dtypes

import concourse.bass as bass
import concourse.tile as tile
from concourse import bacc, mybir
from concourse.bass_utils import run_bass_kernel_spmd

F32 = mybir.dt.float32
BF16 = mybir.dt.bfloat16

B, S, E, H, R = 4, 4096, 1024, 16, 512
D = E // H  # 64
N_CORES = 8


def build_nc(T, n_cores, groups, debug_dump=False):
    """Build the SPMD bass kernel for T tokens per core."""
    CH = min(512, T)        # tokens per chunk
    NCH = T // CH           # chunks
    TB = CH // 128          # 128-token blocks per chunk
    EC = E // 128           # 8 feature chunks of 128
    RC = R // 128           # 4
    FC5 = E // 512          # 2 (512-wide f chunks)
    HP = H // 2             # head pairs
    hpf = 512 // D          # heads per 512-wide chunk = 8

    nc = bacc.Bacc("TRN2", target_bir_lowering=False, debug=False,
                   num_devices=n_cores)

    xq = nc.declare_dram_parameter("xq", [E, T], BF16, isOutput=False).ap()
    xk = nc.declare_dram_parameter("xk", [E, T], BF16, isOutput=False).ap()
    xv = nc.declare_dram_parameter("xv", [E, T], BF16, isOutput=False).ap()
    # host passes W.T for every weight
    qdT = nc.declare_dram_parameter("qdT", [E, R], BF16, isOutput=False).ap()
    kdT = nc.declare_dram_parameter("kdT", [E, R], BF16, isOutput=False).ap()
    vdT = nc.declare_dram_parameter("vdT", [E, R], BF16, isOutput=False).ap()
    quT = nc.declare_dram_parameter("quT", [R, E], BF16, isOutput=False).ap()
    kuT = nc.declare_dram_parameter("kuT", [R, E], BF16, isOutput=False).ap()
    vuT = nc.declare_dram_parameter("vuT", [R, E], BF16, isOutput=False).ap()
    owT = nc.declare_dram_parameter("owT", [E, E], BF16, isOutput=False).ap()
    qu_b = nc.declare_dram_parameter("qu_b", [E], F32, isOutput=False).ap()
    ku_b = nc.declare_dram_parameter("ku_b", [E], BF16, isOutput=False).ap()
    vu_b = nc.declare_dram_parameter("vu_b", [E], BF16, isOutput=False).ap()
    out_b = nc.declare_dram_parameter("out_b", [E], BF16, isOutput=False).ap()
    # host-built broadcast pattern: ppat[h, hp*128+p] = 1 iff head h covers
    # partition p of pair hp (p<64 -> even head, p>=64 -> odd head)
    ppat_d = nc.declare_dram_parameter("ppat", [16, HP * 128], BF16,
                                       isOutput=False).ap()
    y = nc.declare_dram_parameter("y", [T, E], F32, isOutput=True).ap()

    # kv | k_sum buffers: head pair hp -> partitions [0:64] = head 2*hp,
    # [64:128] = head 2*hp+1
    cc_in = nc.dram_tensor("cc_in", [128, HP, D + 1], F32).ap()
    cc_out = nc.dram_tensor("cc_out", [128, HP, D + 1], F32).ap()

    dbg = {}
    if debug_dump:
        for nm, shp in (("qd", [128, RC, T]), ("ktm", [128, TB, H, D + 1]),
                        ("vtm", [128, TB, H, D + 1]), ("kvacc", [128, HP, D + 1]),
                        ("kvred", [128, HP, D + 1]), ("qfm", [128, EC, CH]),
                        ("den", [16, CH]), ("rec", [16, CH]),
                        ("nd0", [128, CH]), ("bc0", [128, CH]),
                        ("att", [128, EC, CH])):
            dbg[nm] = nc.declare_dram_parameter(f"dbg_{nm}", shp, F32,
                                                isOutput=True).ap()

    Exp = mybir.ActivationFunctionType.Exp
    Relu = mybir.ActivationFunctionType.Relu
    Copy = mybir.ActivationFunctionType.Copy

    with tile.TileContext(nc) as tc:
        with (
            tc.tile_pool(name="const", bufs=1) as const,
            tc.tile_pool(name="persist", bufs=1) as persist,
            tc.tile_pool(name="w2a", bufs=1) as w2a,
        ):
            ones_row = const.tile([1, 512], BF16)
            nc.vector.memset(ones_row[:], 1.0)

            # broadcast pattern: bc[p,t] = sum_h ppat[h,hp,p] * rec[h,t]
            ppat = const.tile([16, HP, 128], BF16)
            nc.sync.dma_start(
                out=ppat[:], in_=ppat_d.rearrange("h (c p) -> h c p", p=128))

            ku_brow = const.tile([1, E], BF16, tag="kub")
            vu_brow = const.tile([1, E], BF16, tag="vub")
            ob_row = const.tile([1, E], BF16, tag="ob")
            for t_, a_ in ((ku_brow, ku_b), (vu_brow, vu_b), (ob_row, out_b)):
                nc.sync.dma_start(out=t_[:],
                                  in_=a_.rearrange("(o f) -> o f", o=1))
            # qu_b as per-partition columns (feature-major bias)
            qu_bc = const.tile([128, EC], F32, tag="qubc")
            nc.sync.dma_start(out=qu_bc[:],
                              in_=qu_b.rearrange("(c p) -> p c", p=128))
            # vu_b / out_b broadcast along partitions (token-major adds)
            vu_bcast = const.tile([128, E], BF16, tag="vubc")
            nc.gpsimd.partition_broadcast(vu_bcast[:], vu_brow[:])
            ou_bcast = const.tile([128, E], BF16, tag="oubc")
            nc.gpsimd.partition_broadcast(ou_bcast[:], ob_row[:])

            qd_all = persist.tile([128, RC, T], BF16)     # Qd feature-major
            kv_acc = persist.tile([128, HP, D + 1], F32)  # kv | k_sum

            # ---------------- Phase 1 ----------------
            with (
                tc.tile_pool(name="wkv", bufs=1) as wkv,
            ):
                kdT_sb = wkv.tile([128, EC, R], BF16, tag="kdT")
                vdT_sb = wkv.tile([128, EC, R], BF16, tag="vdT")
                kuT_sb = wkv.tile([128, RC, E], BF16, tag="kuT")
                vuT_sb = wkv.tile([128, RC, E], BF16, tag="vuT")
                qdT_sb = wkv.tile([128, EC, R], BF16, tag="qdT")
                quT_sb = w2a.tile([128, RC, E], BF16, tag="quT")
                nc.sync.dma_start(
                    out=qdT_sb[:], in_=qdT.rearrange("(c p) r -> p c r", p=128))
                nc.sync.dma_start(
                    out=kdT_sb[:], in_=kdT.rearrange("(c p) r -> p c r", p=128))
                nc.sync.dma_start(
                    out=vdT_sb[:], in_=vdT.rearrange("(c p) r -> p c r", p=128))
                nc.sync.dma_start(
                    out=kuT_sb[:], in_=kuT.rearrange("(c p) e -> p c e", p=128))
                nc.sync.dma_start(
                    out=vuT_sb[:], in_=vuT.rearrange("(c p) e -> p c e", p=128))
                nc.sync.dma_start(
                    out=quT_sb[:], in_=quT.rearrange("(c p) e -> p c e", p=128))

                with (
                    tc.tile_pool(name="xfm", bufs=2) as xfmp,
                    tc.tile_pool(name="dpsum", bufs=2, space="PSUM") as dpsum,
                ):
                    def load_x_fm(x_ap, c):
                        xfm = xfmp.tile([128, EC, CH], BF16, tag="xfm")
                        nc.sync.dma_start(
                            out=xfm[:],
                            in_=x_ap.rearrange("(f p) t -> p f t", p=128)
                                    [:, :, c * CH:(c + 1) * CH])
                        return xfm

                    def down_proj(xfm, dwT, dst, dst_sl, on_scalar=False):
                        # dst[:, rb, dst_sl] = (W_down @ x^T) feature-major
                        for rb in range(RC):
                            ps = dpsum.tile([128, CH], F32, tag="dps")
                            for ec in range(EC):
                                nc.tensor.matmul(
                                    ps[:], dwT[:, ec, rb * 128:(rb + 1) * 128],
                                    xfm[:, ec, :],
                                    start=(ec == 0), stop=(ec == EC - 1))
                            if on_scalar:
                                nc.scalar.activation(dst[:, rb, dst_sl],
                                                     ps[:], Copy)
                            else:
                                nc.vector.tensor_copy(dst[:, rb, dst_sl],
                                                      ps[:])

                    # --- 1a: k/v down+up, elu, kv accumulation (first, so
                    # the AllReduce overlaps the q down-projection) ---
                    with (
                        tc.tile_pool(name="dd", bufs=2) as ddp,
                        tc.tile_pool(name="upsum", bufs=2,
                                     space="PSUM") as upsum,
                        tc.tile_pool(name="ktm", bufs=2) as ktmp,
                        tc.tile_pool(name="vtm", bufs=2) as vtmp,
                        tc.tile_pool(name="elu1", bufs=2) as elu1,
                        tc.tile_pool(name="kvps", bufs=4,
                                     space="PSUM") as kvps,
                    ):

                        def up_k(dd, ktm):
                            for tb in range(TB):
                                for fc in range(FC5):
                                    ps = upsum.tile([128, 512], F32,
                                                    tag="ups")
                                    for rc in range(RC):
                                        nc.tensor.matmul(
                                            ps[:],
                                            dd[:, rc,
                                               tb * 128:(tb + 1) * 128],
                                            kuT_sb[:, rc,
                                                   fc * 512:(fc + 1) * 512],
                                            start=(rc == 0), stop=False)
                                    nc.tensor.matmul(
                                        ps[:], ones_row[:, 0:128],
                                        ku_brow[:, fc * 512:(fc + 1) * 512],
                                        start=False, stop=True)
                                    m = elu1.tile([128, 512], F32, tag="m")
                                    nc.vector.tensor_scalar_min(
                                        m[:], ps[:], 0.0)
                                    e = elu1.tile([128, 512], F32, tag="e")
                                    nc.scalar.activation(e[:], m[:], Exp)
                                    r = elu1.tile([128, 512], F32, tag="r")
                                    nc.scalar.activation(r[:], ps[:], Relu)
                                    dsl = ktm[:, tb,
                                              fc * hpf:(fc + 1) * hpf, 0:D]
                                    nc.gpsimd.tensor_add(
                                        dsl,
                                        e[:].rearrange("p (h f) -> p h f",
                                                       h=hpf),
                                        r[:].rearrange("p (h f) -> p h f",
                                                       h=hpf))

                        def up_v(dd, vtm):
                            for tb in range(TB):
                                for fc in range(FC5):
                                    ps = upsum.tile([128, 512], F32,
                                                    tag="ups")
                                    for rc in range(RC):
                                        nc.tensor.matmul(
                                            ps[:],
                                            dd[:, rc,
                                               tb * 128:(tb + 1) * 128],
                                            vuT_sb[:, rc,
                                                   fc * 512:(fc + 1) * 512],
                                            start=(rc == 0),
                                            stop=(rc == RC - 1))
                                    dsl = vtm[:, tb,
                                              fc * hpf:(fc + 1) * hpf, 0:D]
                                    bsl = vu_bcast[:, fc * 512:(fc + 1) * 512]
                                    nc.vector.tensor_add(
                                        dsl,
                                        ps[:].rearrange("p (h f) -> p h f",
                                                        h=hpf),
                                        bsl.rearrange("p (h f) -> p h f",
                                                      h=hpf))

                        for c in range(NCH):
                            xfm = load_x_fm(xk, c)
                            ddk = ddp.tile([128, RC, CH], BF16, tag="dd")
                            down_proj(xfm, kdT_sb, ddk, slice(None),
                                      on_scalar=True)
                            ktm = ktmp.tile([128, TB, H, D + 1], BF16,
                                            tag="ktm")
                            up_k(ddk, ktm)

                            xfm = load_x_fm(xv, c)
                            ddv = ddp.tile([128, RC, CH], BF16, tag="dd")
                            down_proj(xfm, vdT_sb, ddv, slice(None),
                                      on_scalar=True)
                            vtm = vtmp.tile([128, TB, H, D + 1], BF16,
                                            tag="vtm")
                            nc.vector.memset(vtm[:, :, :, D:D + 1], 1.0)
                            up_v(ddv, vtm)

                            if debug_dump and c == 0:
                                nc.gpsimd.dma_start(out=dbg["ktm"][:],
                                                    in_=ktm[:])
                                nc.gpsimd.dma_start(out=dbg["vtm"][:],
                                                    in_=vtm[:])

                            for hp in range(HP):
                                pkv = kvps.tile([128, D + 1], F32, tag="kvps")
                                for tb in range(TB):
                                    nc.tensor.matmul(
                                        pkv[0:64, :],
                                        ktm[:, tb, 2 * hp, 0:D],
                                        vtm[:, tb, 2 * hp, 0:D + 1],
                                        start=(tb == 0), stop=(tb == TB - 1))
                                for tb in range(TB):
                                    nc.tensor.matmul(
                                        pkv[64:128, :],
                                        ktm[:, tb, 2 * hp + 1, 0:D],
                                        vtm[:, tb, 2 * hp + 1, 0:D + 1],
                                        start=(tb == 0), stop=(tb == TB - 1),
                                        tile_position=(0, 64))
                                acc_sl = kv_acc[:, hp, :]
                                if c == 0:
                                    nc.vector.tensor_copy(acc_sl, pkv[:])
                                else:
                                    nc.vector.tensor_add(acc_sl, acc_sl,
                                                         pkv[:])
                        if debug_dump:
                            nc.sync.dma_start(out=dbg["kvacc"][:],
                                              in_=kv_acc[:])

                    # ------- AllReduce kv across the batch pair (overlaps
                    # the q down-projection below) -------
                    nc.sync.dma_start(out=cc_in[:], in_=kv_acc[:])
                    nc.gpsimd.collective_compute(
                        "AllReduce", mybir.AluOpType.add,
                        ins=[cc_in[:]], outs=[cc_out[:]],
                        replica_groups=groups)

                    # --- 1b: q down-projection ---
                    for c in range(NCH):
                        xfm = load_x_fm(xq, c)
                        down_proj(xfm, qdT_sb, qd_all, bass.ds(c * CH, CH))
                    if debug_dump:
                        nc.gpsimd.dma_start(out=dbg["qd"][:], in_=qd_all[:])

            # ---------------- Phase 2 ----------------
            with (
                tc.tile_pool(name="w2", bufs=1) as w2,
                tc.tile_pool(name="kvx", bufs=1) as kvx,
                tc.tile_pool(name="qps", bufs=2, space="PSUM") as qps,
                tc.tile_pool(name="denp", bufs=1, space="PSUM") as denp,
                tc.tile_pool(name="ndp", bufs=2, space="PSUM") as ndp,
                tc.tile_pool(name="bcp", bufs=1, space="PSUM") as bcp,
                tc.tile_pool(name="yps", bufs=2, space="PSUM") as yps,
                tc.tile_pool(name="qfm", bufs=2) as qfmp,
                tc.tile_pool(name="att", bufs=2) as attp,
                tc.tile_pool(name="rec", bufs=2) as recp,
                tc.tile_pool(name="elu2", bufs=2) as elu2,
                tc.tile_pool(name="ysb", bufs=2) as ysbp,
            ):
                owT_sb = w2.tile([128, EC, E], BF16, tag="owT")
                nc.sync.dma_start(
                    out=owT_sb[:], in_=owT.rearrange("(c p) e -> p c e",
                                                     p=128))

                kv_red = kvx.tile([128, HP, D + 1], F32, tag="kvred")
                nc.sync.dma_start(out=kv_red[:], in_=cc_out[:])
                kv_ext = kvx.tile([128, HP, D + 1], BF16, tag="kvext")
                nc.vector.tensor_copy(kv_ext[:], kv_red[:])
                if debug_dump:
                    nc.sync.dma_start(out=dbg["kvred"][:], in_=kv_red[:])

                # KSmat[p, ec, h] = k_sum[h, p - 64*(h%2)] for h//2==ec else 0
                KSmat = kvx.tile([128, EC, 16], BF16, tag="ksmat")
                nc.vector.memset(KSmat[:], 0.0)
                for h in range(H):
                    base = 64 * (h % 2)
                    nc.vector.tensor_copy(
                        KSmat[base:base + 64, h // 2, h:h + 1],
                        kv_ext[base:base + 64, h // 2, D:D + 1])

                for c in range(NCH):
                    # q up-projection (feature-major) + bias + elu+1
                    qfm = qfmp.tile([128, EC, CH], BF16, tag="qfm")
                    for fc in range(EC):
                        ps = qps.tile([128, CH], F32, tag="qps")
                        for rc in range(RC):
                            nc.tensor.matmul(
                                ps[:], quT_sb[:, rc, fc * 128:(fc + 1) * 128],
                                qd_all[:, rc, bass.ds(c * CH, CH)],
                                start=(rc == 0), stop=(rc == RC - 1))
                        m = elu2.tile([128, CH], F32, tag="m2")
                        nc.vector.tensor_scalar(
                            m[:], ps[:], qu_bc[:, fc:fc + 1], 0.0,
                            op0=mybir.AluOpType.add, op1=mybir.AluOpType.min)
                        e = elu2.tile([128, CH], F32, tag="e2")
                        nc.scalar.activation(e[:], m[:], Exp)
                        r = elu2.tile([128, CH], F32, tag="r2")
                        nc.scalar.activation(r[:], ps[:], Relu,
                                             bias=qu_bc[:, fc:fc + 1])
                        nc.gpsimd.tensor_add(qfm[:, fc, :], e[:], r[:])

                    # denominators for all 16 heads at once
                    den = denp.tile([16, CH], F32, tag="den")
                    for ec in range(EC):
                        nc.tensor.matmul(den[:], KSmat[:, ec, :],
                                         qfm[:, ec, :],
                                         start=(ec == 0), stop=(ec == EC - 1))
                    rec = recp.tile([16, CH], F32, tag="rec")
                    nc.vector.reciprocal_approx_fast(rec[:], den[:])
                    recb = recp.tile([16, CH], BF16, tag="recb")
                    nc.vector.tensor_copy(recb[:], rec[:])
                    if debug_dump and c == 0:
                        dent = recp.tile([16, CH], F32, tag="dent")
                        nc.vector.tensor_copy(dent[:], den[:])
                        nc.sync.dma_start(out=dbg["den"][:], in_=dent[:])
                        nc.sync.dma_start(out=dbg["rec"][:], in_=rec[:])
                        nc.gpsimd.dma_start(out=dbg["qfm"][:], in_=qfm[:])

                    # attention per head pair: packed num matmuls + bcast mul
                    att = attp.tile([128, EC, CH], BF16, tag="att")
                    for hp in range(HP):
                        nd = ndp.tile([128, CH], F32, tag="nd")
                        nc.tensor.matmul(nd[0:64, :], kv_ext[0:64, hp, 0:D],
                                         qfm[0:64, hp, :],
                                         start=True, stop=True)
                        nc.tensor.matmul(nd[64:128, :],
                                         kv_ext[64:128, hp, 0:D],
                                         qfm[64:128, hp, :],
                                         start=True, stop=True,
                                         tile_position=(64, 64))
                        bc = bcp.tile([128, CH], F32, tag="bc")
                        nc.tensor.matmul(bc[:], ppat[:, hp, :], recb[:],
                                         start=True, stop=True)
                        bcs = elu2.tile([128, CH], F32, tag="bcs")
                        nc.scalar.activation(bcs[:], bc[:], Copy)
                        nc.vector.tensor_mul(att[:, hp, :], nd[:], bcs[:])
                        if debug_dump and c == 0 and hp == 0:
                            ndt = elu2.tile([128, CH], F32, tag="ndt")
                            nc.vector.tensor_copy(ndt[:], nd[:])
                            nc.sync.dma_start(out=dbg["nd0"][:], in_=ndt[:])
                            nc.sync.dma_start(out=dbg["bc0"][:], in_=bcs[:])

                    if debug_dump and c == 0:
                        nc.gpsimd.dma_start(out=dbg["att"][:], in_=att[:])

                    # output projection (token-major) + bias
                    ysb = ysbp.tile([128, TB, E], F32, tag="ysb")
                    for tb in range(TB):
                        for fo in range(FC5):
                            py = yps.tile([128, 512], F32, tag="yps")
                            for ec in range(EC):
                                nc.tensor.matmul(
                                    py[:],
                                    att[:, ec, tb * 128:(tb + 1) * 128],
                                    owT_sb[:, ec, fo * 512:(fo + 1) * 512],
                                    start=(ec == 0), stop=(ec == EC - 1))
                            nc.vector.tensor_add(
                                ysb[:, tb, fo * 512:(fo + 1) * 512], py[:],
                                ou_bcast[:, fo * 512:(fo + 1) * 512])
                    nc.sync.dma_start(
                        out=y.rearrange("(cc tb p) e -> p cc tb e",
                                        p=128, tb=TB)[:, c, :, :],
                        in_=ysb[:])

    nc.compile()
    return nc


_NC_CACHE = {}


def _get_nc(T, n_cores, groups):
    key = (T, n_cores, tuple(tuple(g) for g in groups))
    if key not in _NC_CACHE:
        _NC_CACHE[key] = build_nc(T, n_cores, groups)
    return _NC_CACHE[key]


def _make_in_maps(inputs):
    bf = ml_dtypes.bfloat16
    query = np.asarray(inputs["query"], dtype=np.float32)
    key = np.asarray(inputs["key"], dtype=np.float32)
    value = np.asarray(inputs["value"], dtype=np.float32)

    weights = {
        "qdT": np.asarray(inputs["qd_w"], np.float32).T.astype(bf),
        "kdT": np.asarray(inputs["kd_w"], np.float32).T.astype(bf),
        "vdT": np.asarray(inputs["vd_w"], np.float32).T.astype(bf),
        "quT": np.asarray(inputs["qu_w"], np.float32).T.astype(bf),
        "kuT": np.asarray(inputs["ku_w"], np.float32).T.astype(bf),
        "vuT": np.asarray(inputs["vu_w"], np.float32).T.astype(bf),
        "owT": np.asarray(inputs["out_w"], np.float32).T.astype(bf),
        "qu_b": np.asarray(inputs["qu_b"], np.float32),
        "ku_b": np.asarray(inputs["ku_b"], np.float32).astype(bf),
        "vu_b": np.asarray(inputs["vu_b"], np.float32).astype(bf),
        "out_b": np.asarray(inputs["out_b"], np.float32).astype(bf),
    }
    HP = H // 2
    ppat = np.zeros((16, HP * 128), dtype=np.float32)
    for hp in range(HP):
        ppat[2 * hp, hp * 128:hp * 128 + 64] = 1.0
        ppat[2 * hp + 1, hp * 128 + 64:hp * 128 + 128] = 1.0
    weights["ppat"] = ppat.astype(bf)

    half = S // 2
    in_maps = []
    for c in range(N_CORES):
        bi, hi = c // 2, c % 2
        sl = slice(hi * half, (hi + 1) * half)
        m = {
            "xq": query[bi, sl].T.astype(bf),
            "xk": key[bi, sl].T.astype(bf),
            "xv": value[bi, sl].T.astype(bf),
        }
        m.update(weights)
        in_maps.append(m)
    return in_maps


def kernel(**inputs):
    b, s, e = np.asarray(inputs["query"]).shape
    assert (b, s, e) == (B, S, E)

    T = B * S // N_CORES  # 2048 tokens per core
    half = S // 2
    groups = [[0, 1], [2, 3], [4, 5], [6, 7]]
    nc = _get_nc(T, N_CORES, groups)

    in_maps = _make_in_maps(inputs)
    res = run_bass_kernel_spmd(nc, in_maps, list(range(N_CORES)))

    out = np.empty((B, S, E), dtype=np.float32)
    for c in range(N_CORES):
        bi, hi = c // 2, c % 2
        out[bi, hi * half:(hi + 1) * half] = res.results[c]["y"]
    return out


# revision 45
# speedup vs baseline: 1.1761x; 1.0215x over previous
"""Trainium2 Bass kernel for nn_CustomMultiheadAttention (linear attention with
low-rank QKV projections), SPMD over 8 NeuronCores.

Sharding: (batch, seq-half) -> core.  Core c handles batch c//2, sequence half
c%2 (2048 tokens).  Pairwise AllReduce of kv/k_sum between the two cores
sharing a batch.

Host-side prep: x and all weights are pre-transposed and cast to bf16 in
numpy so the device sees feature-major operands directly (no PE transposes,
no cast copies).  Bias adds are folded into the matmul accumulation groups as
K=1 ones-vector matmuls.  Attention denominators are accumulated for all 16
heads at once via a [128,16] selection-matrix matmul, inverted with one
reciprocal_approx_fast per chunk, and broadcast back with a K=16 pattern
matmul; head pairs share one [128,CH] PSUM tile so the divide is a single
tensor_tensor per pair.

elu(z)+1 is computed as exp(min(z,0)) + relu(z)  (exact).
"""

import numpy as np
import ml_dtypes

import concourse.bass as bass
import concourse.tile as tile
from concourse import bacc, mybir
from concourse.bass_utils import run_bass_kernel_spmd

F32 = mybir.dt.float32
BF16 = mybir.dt.bfloat16

B, S, E, H, R = 4, 4096, 1024, 16, 512
D = E // H  # 64
N_CORES = 8


def build_nc(T, n_cores, groups, debug_dump=False):
    """Build the SPMD bass kernel for T tokens per core."""
    CH = min(512, T)        # tokens per chunk
    NCH = T // CH           # chunks
    TB = CH // 128          # 128-token blocks per chunk
    EC = E // 128           # 8 feature chunks of 128
    RC = R // 128           # 4
    FC5 = E // 512          # 2 (512-wide f chunks)
    HP = H // 2             # head pairs
    hpf = 512 // D          # heads per 512-wide chunk = 8

    nc = bacc.Bacc("TRN2", target_bir_lowering=False, debug=False,
                   num_devices=n_cores)

    xq = nc.declare_dram_parameter("xq", [E, T], BF16, isOutput=False).ap()
    xk = nc.declare_dram_parameter("xk", [E, T], BF16, isOutput=False).ap()
    xv = nc.declare_dram_parameter("xv", [E, T], BF16, isOutput=False).ap()
    # host passes W.T for every weight
    qdT = nc.declare_dram_parameter("qdT", [E, R], BF16, isOutput=False).ap()
    kdT = nc.declare_dram_parameter("kdT", [E, R], BF16, isOutput=False).ap()
    vdT = nc.declare_dram_parameter("vdT", [E, R], BF16, isOutput=False).ap()
    quT = nc.declare_dram_parameter("quT", [R, E], BF16, isOutput=False).ap()
    kuT = nc.declare_dram_parameter("kuT", [R, E], BF16, isOutput=False).ap()
    vuT = nc.declare_dram_parameter("vuT", [R, E], BF16, isOutput=False).ap()
    owT = nc.declare_dram_parameter("owT", [E, E], BF16, isOutput=False).ap()
    qu_b = nc.declare_dram_parameter("qu_b", [E], F32, isOutput=False).ap()
    ku_b = nc.declare_dram_parameter("ku_b", [E], BF16, isOutput=False).ap()
    vu_b = nc.declare_dram_parameter("vu_b", [E], BF16, isOutput=False).ap()
    out_b = nc.declare_dram_parameter("out_b", [E], BF16, isOutput=False).ap()
    # host-built broadcast pattern: ppat[h, hp*128+p] = 1 iff head h covers
    # partition p of pair hp (p<64 -> even head, p>=64 -> odd head)
    ppat_d = nc.declare_dram_parameter("ppat", [16, HP * 128], BF16,
                                       isOutput=False).ap()
    y = nc.declare_dram_parameter("y", [T, E], F32, isOutput=True).ap()

    # kv | k_sum buffers: head pair hp -> partitions [0:64] = head 2*hp,
    # [64:128] = head 2*hp+1
    cc_in = nc.dram_tensor("cc_in", [128, HP, D + 1], F32).ap()
    cc_out = nc.dram_tensor("cc_out", [128, HP, D + 1], F32).ap()

    dbg = {}
    if debug_dump:
        for nm, shp in (("qd", [128, RC, T]), ("ktm", [128, TB, H, D + 1]),
                        ("vtm", [128, TB, H, D + 1]), ("kvacc", [128, HP, D + 1]),
                        ("kvred", [128, HP, D + 1]), ("qfm", [128, EC, CH]),
                        ("den", [16, CH]), ("rec", [16, CH]),
                        ("nd0", [128, CH]), ("bc0", [128, CH]),
                        ("att", [128, EC, CH])):
            dbg[nm] = nc.declare_dram_parameter(f"dbg_{nm}", shp, F32,
                                                isOutput=True).ap()

    Exp = mybir.ActivationFunctionType.Exp
    Relu = mybir.ActivationFunctionType.Relu
    Copy = mybir.ActivationFunctionType.Copy

    with tile.TileContext(nc) as tc:
        with (
            tc.tile_pool(name="const", bufs=1) as const,
            tc.tile_pool(name="persist", bufs=1) as persist,
            tc.tile_pool(name="w2a", bufs=1) as w2a,
        ):
            ones_row = const.tile([1, 512], BF16)
            nc.vector.memset(ones_row[:], 1.0)

            # broadcast pattern: bc[p,t] = sum_h ppat[h,hp,p] * rec[h,t]
            ppat = const.tile([16, HP, 128], BF16)
            nc.sync.dma_start(
                out=ppat[:], in_=ppat_d.rearrange("h (c p) -> h c p", p=128))

            ku_brow = const.tile([1, E], BF16, tag="kub")
            vu_brow = const.tile([1, E], BF16, tag="vub")
            ob_row = const.tile([1, E], BF16, tag="ob")
            for t_, a_ in ((ku_brow, ku_b), (vu_brow, vu_b), (ob_row, out_b)):
                nc.sync.dma_start(out=t_[:],
                                  in_=a_.rearrange("(o f) -> o f", o=1))
            # qu_b as per-partition columns (feature-major bias)
            qu_bc = const.tile([128, EC], F32, tag="qubc")
            nc.sync.dma_start(out=qu_bc[:],
                              in_=qu_b.rearrange("(c p) -> p c", p=128))
            # vu_b / out_b broadcast along partitions (token-major adds)
            vu_bcast = const.tile([128, E], BF16, tag="vubc")
            nc.gpsimd.partition_broadcast(vu_bcast[:], vu_brow[:])
            ou_bcast = const.tile([128, E], BF16, tag="oubc")
            nc.gpsimd.partition_broadcast(ou_bcast[:], ob_row[:])

            qd_all = persist.tile([128, RC, T], BF16)     # Qd feature-major
            kv_acc = persist.tile([128, HP, D + 1], F32)  # kv | k_sum

            # ---------------- Phase 1 ----------------
            with (
                tc.tile_pool(name="wkv", bufs=1) as wkv,
            ):
                kdT_sb = wkv.tile([128, EC, R], BF16, tag="kdT")
                vdT_sb = wkv.tile([128, EC, R], BF16, tag="vdT")
                kuT_sb = wkv.tile([128, RC, E], BF16, tag="kuT")
                vuT_sb = wkv.tile([128, RC, E], BF16, tag="vuT")
                qdT_sb = wkv.tile([128, EC, R], BF16, tag="qdT")
                quT_sb = w2a.tile([128, RC, E], BF16, tag="quT")
                # spread weight loads across engine DMA queues so the
                # first k-chunk x load is not stuck behind 8MB of weights
                nc.sync.dma_start(
                    out=kdT_sb[:], in_=kdT.rearrange("(c p) r -> p c r", p=128))
                nc.scalar.dma_start(
                    out=vdT_sb[:], in_=vdT.rearrange("(c p) r -> p c r", p=128))
                nc.scalar.dma_start(
                    out=kuT_sb[:], in_=kuT.rearrange("(c p) e -> p c e", p=128))
                nc.scalar.dma_start(
                    out=vuT_sb[:], in_=vuT.rearrange("(c p) e -> p c e", p=128))
                nc.scalar.dma_start(
                    out=qdT_sb[:], in_=qdT.rearrange("(c p) r -> p c r", p=128))
                nc.scalar.dma_start(
                    out=quT_sb[:], in_=quT.rearrange("(c p) e -> p c e", p=128))

                with (
                    tc.tile_pool(name="xfm", bufs=2) as xfmp,
                    tc.tile_pool(name="dpsum", bufs=2, space="PSUM") as dpsum,
                ):
                    def load_x_fm(x_ap, c):
                        xfm = xfmp.tile([128, EC, CH], BF16, tag="xfm")
                        nc.sync.dma_start(
                            out=xfm[:],
                            in_=x_ap.rearrange("(f p) t -> p f t", p=128)
                                    [:, :, c * CH:(c + 1) * CH])
                        return xfm

                    def down_proj(xfm, dwT, dst, dst_sl, on_scalar=False):
                        # dst[:, rb, dst_sl] = (W_down @ x^T) feature-major
                        for rb in range(RC):
                            ps = dpsum.tile([128, CH], F32, tag="dps")
                            for ec in range(EC):
                                nc.tensor.matmul(
                                    ps[:], dwT[:, ec, rb * 128:(rb + 1) * 128],
                                    xfm[:, ec, :],
                                    start=(ec == 0), stop=(ec == EC - 1))
                            if on_scalar:
                                nc.scalar.activation(dst[:, rb, dst_sl],
                                                     ps[:], Copy)
                            else:
                                nc.vector.tensor_copy(dst[:, rb, dst_sl],
                                                      ps[:])

                    # --- 1a: k/v down+up, elu, kv accumulation (first, so
                    # the AllReduce overlaps the q down-projection) ---
                    with (
                        tc.tile_pool(name="dd", bufs=2) as ddp,
                        tc.tile_pool(name="upsum", bufs=2,
                                     space="PSUM") as upsum,
                        tc.tile_pool(name="ktm", bufs=2) as ktmp,
                        tc.tile_pool(name="vtm", bufs=2) as vtmp,
                        tc.tile_pool(name="elu1", bufs=2) as elu1,
                        tc.tile_pool(name="kvps", bufs=4,
                                     space="PSUM") as kvps,
                    ):

                        def up_k(dd, ktm):
                            for tb in range(TB):
                                for fc in range(FC5):
                                    ps = upsum.tile([128, 512], F32,
                                                    tag="ups")
                                    for rc in range(RC):
                                        nc.tensor.matmul(
                                            ps[:],
                                            dd[:, rc,
                                               tb * 128:(tb + 1) * 128],
                                            kuT_sb[:, rc,
                                                   fc * 512:(fc + 1) * 512],
                                            start=(rc == 0), stop=False)
                                    nc.tensor.matmul(
                                        ps[:], ones_row[:, 0:128],
                                        ku_brow[:, fc * 512:(fc + 1) * 512],
                                        start=False, stop=True)
                                    m = elu1.tile([128, 512], F32, tag="m")
                                    nc.vector.tensor_scalar_min(
                                        m[:], ps[:], 0.0)
                                    e = elu1.tile([128, 512], F32, tag="e")
                                    nc.scalar.activation(e[:], m[:], Exp)
                                    r = elu1.tile([128, 512], F32, tag="r")
                                    nc.scalar.activation(r[:], ps[:], Relu)
                                    dsl = ktm[:, tb,
                                              fc * hpf:(fc + 1) * hpf, 0:D]
                                    nc.vector.tensor_add(
                                        dsl,
                                        e[:].rearrange("p (h f) -> p h f",
                                                       h=hpf),
                                        r[:].rearrange("p (h f) -> p h f",
                                                       h=hpf))

                        def up_v(dd, vtm):
                            for tb in range(TB):
                                for fc in range(FC5):
                                    ps = upsum.tile([128, 512], F32,
                                                    tag="ups")
                                    for rc in range(RC):
                                        nc.tensor.matmul(
                                            ps[:],
                                            dd[:, rc,
                                               tb * 128:(tb + 1) * 128],
                                            vuT_sb[:, rc,
                                                   fc * 512:(fc + 1) * 512],
                                            start=(rc == 0),
                                            stop=(rc == RC - 1))
                                    dsl = vtm[:, tb,
                                              fc * hpf:(fc + 1) * hpf, 0:D]
                                    bsl = vu_bcast[:, fc * 512:(fc + 1) * 512]
                                    nc.vector.tensor_add(
                                        dsl,
                                        ps[:].rearrange("p (h f) -> p h f",
                                                        h=hpf),
                                        bsl.rearrange("p (h f) -> p h f",
                                                      h=hpf))

                        for c in range(NCH):
                            xfm = load_x_fm(xk, c)
                            ddk = ddp.tile([128, RC, CH], BF16, tag="dd")
                            down_proj(xfm, kdT_sb, ddk, slice(None),
                                      on_scalar=True)
                            ktm = ktmp.tile([128, TB, H, D + 1], BF16,
                                            tag="ktm")
                            up_k(ddk, ktm)

                            xfm = load_x_fm(xv, c)
                            ddv = ddp.tile([128, RC, CH], BF16, tag="dd")
                            down_proj(xfm, vdT_sb, ddv, slice(None),
                                      on_scalar=True)
                            vtm = vtmp.tile([128, TB, H, D + 1], BF16,
                                            tag="vtm")
                            nc.vector.memset(vtm[:, :, :, D:D + 1], 1.0)
                            up_v(ddv, vtm)

                            if debug_dump and c == 0:
                                nc.gpsimd.dma_start(out=dbg["ktm"][:],
                                                    in_=ktm[:])
                                nc.gpsimd.dma_start(out=dbg["vtm"][:],
                                                    in_=vtm[:])

                            for hp in range(HP):
                                pkv = kvps.tile([128, D + 1], F32, tag="kvps")
                                for tb in range(TB):
                                    nc.tensor.matmul(
                                        pkv[0:64, :],
                                        ktm[:, tb, 2 * hp, 0:D],
                                        vtm[:, tb, 2 * hp, 0:D + 1],
                                        start=(tb == 0), stop=(tb == TB - 1))
                                for tb in range(TB):
                                    nc.tensor.matmul(
                                        pkv[64:128, :],
                                        ktm[:, tb, 2 * hp + 1, 0:D],
                                        vtm[:, tb, 2 * hp + 1, 0:D + 1],
                                        start=(tb == 0), stop=(tb == TB - 1),
                                        tile_position=(0, 64))
                                acc_sl = kv_acc[:, hp, :]
                                if c == 0:
                                    nc.vector.tensor_copy(acc_sl, pkv[:])
                                else:
                                    nc.vector.tensor_add(acc_sl, acc_sl,
                                                         pkv[:])
                        if debug_dump:
                            nc.sync.dma_start(out=dbg["kvacc"][:],
                                              in_=kv_acc[:])

                    # ------- AllReduce kv across the batch pair (overlaps
                    # the q down-projection below) -------
                    nc.sync.dma_start(out=cc_in[:], in_=kv_acc[:])
                    nc.gpsimd.collective_compute(
                        "AllReduce", mybir.AluOpType.add,
                        ins=[cc_in[:]], outs=[cc_out[:]],
                        replica_groups=groups)

                    # --- 1b: q down-projection ---
                    for c in range(NCH):
                        xfm = load_x_fm(xq, c)
                        down_proj(xfm, qdT_sb, qd_all, bass.ds(c * CH, CH))
                    if debug_dump:
                        nc.gpsimd.dma_start(out=dbg["qd"][:], in_=qd_all[:])

            # ---------------- Phase 2 ----------------
            with (
                tc.tile_pool(name="w2", bufs=1) as w2,
                tc.tile_pool(name="kvx", bufs=1) as kvx,
                tc.tile_pool(name="qps", bufs=2, space="PSUM") as qps,
                tc.tile_pool(name="denp", bufs=1, space="PSUM") as denp,
                tc.tile_pool(name="ndp", bufs=2, space="PSUM") as ndp,
                tc.tile_pool(name="bcp", bufs=1, space="PSUM") as bcp,
                tc.tile_pool(name="yps", bufs=2, space="PSUM") as yps,
                tc.tile_pool(name="qfm", bufs=2) as qfmp,
                tc.tile_pool(name="att", bufs=2) as attp,
                tc.tile_pool(name="rec", bufs=2) as recp,
                tc.tile_pool(name="elu2", bufs=2) as elu2,
                tc.tile_pool(name="ysb", bufs=2) as ysbp,
            ):
                owT_sb = w2.tile([128, EC, E], BF16, tag="owT")
                nc.sync.dma_start(
                    out=owT_sb[:], in_=owT.rearrange("(c p) e -> p c e",
                                                     p=128))

                kv_red = kvx.tile([128, HP, D + 1], F32, tag="kvred")
                nc.sync.dma_start(out=kv_red[:], in_=cc_out[:])
                kv_ext = kvx.tile([128, HP, D + 1], BF16, tag="kvext")
                nc.vector.tensor_copy(kv_ext[:], kv_red[:])
                if debug_dump:
                    nc.sync.dma_start(out=dbg["kvred"][:], in_=kv_red[:])

                # KSmat[p, ec, h] = k_sum[h, p - 64*(h%2)] for h//2==ec else 0
                KSmat = kvx.tile([128, EC, 16], BF16, tag="ksmat")
                nc.vector.memset(KSmat[:], 0.0)
                for h in range(H):
                    base = 64 * (h % 2)
                    nc.vector.tensor_copy(
                        KSmat[base:base + 64, h // 2, h:h + 1],
                        kv_ext[base:base + 64, h // 2, D:D + 1])

                for c in range(NCH):
                    # q up-projection (feature-major) + bias + elu+1
                    qfm = qfmp.tile([128, EC, CH], BF16, tag="qfm")
                    for fc in range(EC):
                        ps = qps.tile([128, CH], F32, tag="qps")
                        for rc in range(RC):
                            nc.tensor.matmul(
                                ps[:], quT_sb[:, rc, fc * 128:(fc + 1) * 128],
                                qd_all[:, rc, bass.ds(c * CH, CH)],
                                start=(rc == 0), stop=(rc == RC - 1))
                        m = elu2.tile([128, CH], F32, tag="m2")
                        nc.vector.tensor_scalar(
                            m[:], ps[:], qu_bc[:, fc:fc + 1], 0.0,
                            op0=mybir.AluOpType.add, op1=mybir.AluOpType.min)
                        e = elu2.tile([128, CH], F32, tag="e2")
                        nc.scalar.activation(e[:], m[:], Exp)
                        r = elu2.tile([128, CH], F32, tag="r2")
                        nc.scalar.activation(r[:], ps[:], Relu,
                                             bias=qu_bc[:, fc:fc + 1])
                        nc.vector.tensor_add(qfm[:, fc, :], e[:], r[:])

                    # denominators for all 16 heads at once
                    den = denp.tile([16, CH], F32, tag="den")
                    for ec in range(EC):
                        nc.tensor.matmul(den[:], KSmat[:, ec, :],
                                         qfm[:, ec, :],
                                         start=(ec == 0), stop=(ec == EC - 1))
                    rec = recp.tile([16, CH], F32, tag="rec")
                    nc.vector.reciprocal_approx_fast(rec[:], den[:])
                    recb = recp.tile([16, CH], BF16, tag="recb")
                    nc.vector.tensor_copy(recb[:], rec[:])
                    if debug_dump and c == 0:
                        dent = recp.tile([16, CH], F32, tag="dent")
                        nc.vector.tensor_copy(dent[:], den[:])
                        nc.sync.dma_start(out=dbg["den"][:], in_=dent[:])
                        nc.sync.dma_start(out=dbg["rec"][:], in_=rec[:])
                        nc.gpsimd.dma_start(out=dbg["qfm"][:], in_=qfm[:])

                    # attention per head pair: packed num matmuls + bcast mul
                    att = attp.tile([128, EC, CH], BF16, tag="att")
                    for hp in range(HP):
                        nd = ndp.tile([128, CH], F32, tag="nd")
                        nc.tensor.matmul(nd[0:64, :], kv_ext[0:64, hp, 0:D],
                                         qfm[0:64, hp, :],
                                         start=True, stop=True)
                        nc.tensor.matmul(nd[64:128, :],
                                         kv_ext[64:128, hp, 0:D],
                                         qfm[64:128, hp, :],
                                         start=True, stop=True,
                                         tile_position=(64, 64))
                        bc = bcp.tile([128, CH], F32, tag="bc")
                        nc.tensor.matmul(bc[:], ppat[:, hp, :], recb[:],
                                         start=True, stop=True)
                        bcs = elu2.tile([128, CH], F32, tag="bcs")
                        nc.scalar.activation(bcs[:], bc[:], Copy)
                        nc.vector.tensor_mul(att[:, hp, :], nd[:], bcs[:])
                        if debug_dump and c == 0 and hp == 0:
                            ndt = elu2.tile([128, CH], F32, tag="ndt")
                            nc.vector.tensor_copy(ndt[:], nd[:])
                            nc.sync.dma_start(out=dbg["nd0"][:], in_=ndt[:])
                            nc.sync.dma_start(out=dbg["bc0"][:], in_=bcs[:])

                    if debug_dump and c == 0:
                        nc.gpsimd.dma_start(out=dbg["att"][:], in_=att[:])

                    # output projection (token-major) + bias
                    for tb in range(TB):
                        ysb = ysbp.tile([128, E], F32, tag="ysb")
                        for fo in range(FC5):
                            py = yps.tile([128, 512], F32, tag="yps")
                            for ec in range(EC):
                                nc.tensor.matmul(
                                    py[:],
                                    att[:, ec, tb * 128:(tb + 1) * 128],
                                    owT_sb[:, ec, fo * 512:(fo + 1) * 512],
                                    start=(ec == 0), stop=(ec == EC - 1))
                            nc.vector.tensor_add(
                                ysb[:, fo * 512:(fo + 1) * 512], py[:],
                                ou_bcast[:, fo * 512:(fo + 1) * 512])
                        r0 = c * CH + tb * 128
                        nc.sync.dma_start(out=y[r0:r0 + 128, :], in_=ysb[:])

    nc.compile()
    return nc


_NC_CACHE = {}


def _get_nc(T, n_cores, groups):
    key = (T, n_cores, tuple(tuple(g) for g in groups))
    if key not in _NC_CACHE:
        _NC_CACHE[key] = build_nc(T, n_cores, groups)
    return _NC_CACHE[key]


def _make_in_maps(inputs):
    bf = ml_dtypes.bfloat16
    query = np.asarray(inputs["query"], dtype=np.float32)
    key = np.asarray(inputs["key"], dtype=np.float32)
    value = np.asarray(inputs["value"], dtype=np.float32)

    weights = {
        "qdT": np.asarray(inputs["qd_w"], np.float32).T.astype(bf),
        "kdT": np.asarray(inputs["kd_w"], np.float32).T.astype(bf),
        "vdT": np.asarray(inputs["vd_w"], np.float32).T.astype(bf),
        "quT": np.asarray(inputs["qu_w"], np.float32).T.astype(bf),
        "kuT": np.asarray(inputs["ku_w"], np.float32).T.astype(bf),
        "vuT": np.asarray(inputs["vu_w"], np.float32).T.astype(bf),
        "owT": np.asarray(inputs["out_w"], np.float32).T.astype(bf),
        "qu_b": np.asarray(inputs["qu_b"], np.float32),
        "ku_b": np.asarray(inputs["ku_b"], np.float32).astype(bf),
        "vu_b": np.asarray(inputs["vu_b"], np.float32).astype(bf),
        "out_b": np.asarray(inputs["out_b"], np.float32).astype(bf),
    }
    HP = H // 2
    ppat = np.zeros((16, HP * 128), dtype=np.float32)
    for hp in range(HP):
        ppat[2 * hp, hp * 128:hp * 128 + 64] = 1.0
        ppat[2 * hp + 1, hp * 128 + 64:hp * 128 + 128] = 1.0
    weights["ppat"] = ppat.astype(bf)

    half = S // 2
    in_maps = []
    for c in range(N_CORES):
        bi, hi = c // 2, c % 2
        sl = slice(hi * half, (hi + 1) * half)
        m = {
            "xq": query[bi, sl].T.astype(bf),
            "xk": key[bi, sl].T.astype(bf),
            "xv": value[bi, sl].T.astype(bf),
        }
        m.update(weights)
        in_maps.append(m)
    return in_maps


def kernel(**inputs):
    b, s, e = np.asarray(inputs["query"]).shape
    assert (b, s, e) == (B, S, E)

    T = B * S // N_CORES  # 2048 tokens per core
    half = S // 2
    groups = [[0, 1], [2, 3], [4, 5], [6, 7]]
    nc = _get_nc(T, N_CORES, groups)

    in_maps = _make_in_maps(inputs)
    res = run_bass_kernel_spmd(nc, in_maps, list(range(N_CORES)))

    out = np.empty((B, S, E), dtype=np.float32)
    for c in range(N_CORES):
        bi, hi = c // 2, c % 2
        out[bi, hi * half:(hi + 1) * half] = res.results[c]["y"]
    return out


# revision 48
# speedup vs baseline: 1.1775x; 1.0012x over previous
"""Trainium2 Bass kernel for nn_CustomMultiheadAttention (linear attention with
low-rank QKV projections), SPMD over 8 NeuronCores.

Sharding: (batch, seq-half) -> core.  Core c handles batch c//2, sequence half
c%2 (2048 tokens).  Pairwise AllReduce of kv/k_sum between the two cores
sharing a batch.

Host-side prep: x and all weights are pre-transposed and cast to bf16 in
numpy so the device sees feature-major operands directly (no PE transposes,
no cast copies).  Bias adds are folded into the matmul accumulation groups as
K=1 ones-vector matmuls.  Attention denominators are accumulated for all 16
heads at once via a [128,16] selection-matrix matmul, inverted with one
reciprocal_approx_fast per chunk, and broadcast back with a K=16 pattern
matmul; head pairs share one [128,CH] PSUM tile so the divide is a single
tensor_tensor per pair.

elu(z)+1 is computed as exp(min(z,0)) + relu(z)  (exact).
"""

import numpy as np
import ml_dtypes

import concourse.bass as bass
import concourse.tile as tile
from concourse import bacc, mybir
from concourse.bass_utils import run_bass_kernel_spmd

F32 = mybir.dt.float32
BF16 = mybir.dt.bfloat16

B, S, E, H, R = 4, 4096, 1024, 16, 512
D = E // H  # 64
N_CORES = 8


def build_nc(T, n_cores, groups, debug_dump=False):
    """Build the SPMD bass kernel for T tokens per core."""
    CH = min(512, T)        # tokens per chunk
    NCH = T // CH           # chunks
    TB = CH // 128          # 128-token blocks per chunk
    EC = E // 128           # 8 feature chunks of 128
    RC = R // 128           # 4
    FC5 = E // 512          # 2 (512-wide f chunks)
    HP = H // 2             # head pairs
    hpf = 512 // D          # heads per 512-wide chunk = 8

    nc = bacc.Bacc("TRN2", target_bir_lowering=False, debug=False,
                   num_devices=n_cores)

    xq = nc.declare_dram_parameter("xq", [E, T], BF16, isOutput=False).ap()
    xk = nc.declare_dram_parameter("xk", [E, T], BF16, isOutput=False).ap()
    xv = nc.declare_dram_parameter("xv", [E, T], BF16, isOutput=False).ap()
    # host passes W.T for every weight
    qdT = nc.declare_dram_parameter("qdT", [E, R], BF16, isOutput=False).ap()
    kdT = nc.declare_dram_parameter("kdT", [E, R], BF16, isOutput=False).ap()
    vdT = nc.declare_dram_parameter("vdT", [E, R], BF16, isOutput=False).ap()
    quT = nc.declare_dram_parameter("quT", [R, E], BF16, isOutput=False).ap()
    kuT = nc.declare_dram_parameter("kuT", [R, E], BF16, isOutput=False).ap()
    vuT = nc.declare_dram_parameter("vuT", [R, E], BF16, isOutput=False).ap()
    owT = nc.declare_dram_parameter("owT", [E, E], BF16, isOutput=False).ap()
    qu_b = nc.declare_dram_parameter("qu_b", [E], F32, isOutput=False).ap()
    ku_b = nc.declare_dram_parameter("ku_b", [E], BF16, isOutput=False).ap()
    vu_b = nc.declare_dram_parameter("vu_b", [E], BF16, isOutput=False).ap()
    out_b = nc.declare_dram_parameter("out_b", [E], BF16, isOutput=False).ap()
    # host-built broadcast pattern: ppat[h, hp*128+p] = 1 iff head h covers
    # partition p of pair hp (p<64 -> even head, p>=64 -> odd head)
    ppat_d = nc.declare_dram_parameter("ppat", [16, HP * 128], BF16,
                                       isOutput=False).ap()
    y = nc.declare_dram_parameter("y", [T, E], F32, isOutput=True).ap()

    # kv | k_sum buffers: head pair hp -> partitions [0:64] = head 2*hp,
    # [64:128] = head 2*hp+1
    cc_in = nc.dram_tensor("cc_in", [128, HP, D + 1], F32).ap()
    cc_out = nc.dram_tensor("cc_out", [128, HP, D + 1], F32).ap()

    dbg = {}
    if debug_dump:
        for nm, shp in (("qd", [128, RC, T]), ("ktm", [128, TB, H, D + 1]),
                        ("vtm", [128, TB, H, D + 1]), ("kvacc", [128, HP, D + 1]),
                        ("kvred", [128, HP, D + 1]), ("qfm", [128, EC, CH]),
                        ("den", [16, CH]), ("rec", [16, CH]),
                        ("nd0", [128, CH]), ("bc0", [128, CH]),
                        ("att", [128, EC, CH])):
            dbg[nm] = nc.declare_dram_parameter(f"dbg_{nm}", shp, F32,
                                                isOutput=True).ap()

    Exp = mybir.ActivationFunctionType.Exp
    Relu = mybir.ActivationFunctionType.Relu
    Copy = mybir.ActivationFunctionType.Copy

    with tile.TileContext(nc) as tc:
        with (
            tc.tile_pool(name="const", bufs=1) as const,
            tc.tile_pool(name="persist", bufs=1) as persist,
            tc.tile_pool(name="w2a", bufs=1) as w2a,
        ):
            ones_row = const.tile([1, 512], BF16)
            nc.vector.memset(ones_row[:], 1.0)

            # broadcast pattern: bc[p,t] = sum_h ppat[h,hp,p] * rec[h,t]
            ppat = const.tile([16, HP, 128], BF16)
            nc.sync.dma_start(
                out=ppat[:], in_=ppat_d.rearrange("h (c p) -> h c p", p=128))

            ku_brow = const.tile([1, E], BF16, tag="kub")
            vu_brow = const.tile([1, E], BF16, tag="vub")
            ob_row = const.tile([1, E], BF16, tag="ob")
            for t_, a_ in ((ku_brow, ku_b), (vu_brow, vu_b), (ob_row, out_b)):
                nc.sync.dma_start(out=t_[:],
                                  in_=a_.rearrange("(o f) -> o f", o=1))
            # qu_b as per-partition columns (feature-major bias)
            qu_bc = const.tile([128, EC], F32, tag="qubc")
            nc.sync.dma_start(out=qu_bc[:],
                              in_=qu_b.rearrange("(c p) -> p c", p=128))
            # vu_b / out_b broadcast along partitions (token-major adds)
            vu_bcast = const.tile([128, E], BF16, tag="vubc")
            nc.gpsimd.partition_broadcast(vu_bcast[:], vu_brow[:])
            ou_bcast = const.tile([128, E], BF16, tag="oubc")
            nc.gpsimd.partition_broadcast(ou_bcast[:], ob_row[:])

            qd_all = persist.tile([128, RC, T], BF16)     # Qd feature-major
            kv_acc = persist.tile([128, HP, D + 1], F32)  # kv | k_sum

            # ---------------- Phase 1 ----------------
            with (
                tc.tile_pool(name="wkv", bufs=1) as wkv,
            ):
                kdT_sb = wkv.tile([128, EC, R], BF16, tag="kdT")
                vdT_sb = wkv.tile([128, EC, R], BF16, tag="vdT")
                kuT_sb = wkv.tile([128, RC, E], BF16, tag="kuT")
                vuT_sb = wkv.tile([128, RC, E], BF16, tag="vuT")
                qdT_sb = wkv.tile([128, EC, R], BF16, tag="qdT")
                quT_sb = w2a.tile([128, RC, E], BF16, tag="quT")
                # critical-path first: the opening k-chunk needs only kdT
                # (sync queue) + xk chunk 0 (scalar queue, issued below);
                # everything else queues behind them
                nc.sync.dma_start(
                    out=kdT_sb[:], in_=kdT.rearrange("(c p) r -> p c r", p=128))

                with (
                    tc.tile_pool(name="xfm", bufs=2) as xfmp,
                    tc.tile_pool(name="dpsum", bufs=2, space="PSUM") as dpsum,
                ):
                    def load_x_fm(x_ap, c, eng=None):
                        xfm = xfmp.tile([128, EC, CH], BF16, tag="xfm")
                        (eng or nc.sync).dma_start(
                            out=xfm[:],
                            in_=x_ap.rearrange("(f p) t -> p f t", p=128)
                                    [:, :, c * CH:(c + 1) * CH])
                        return xfm

                    # opening x chunk on the scalar queue, ahead of the
                    # remaining weight loads
                    xk0 = load_x_fm(xk, 0, eng=nc.scalar)
                    nc.scalar.dma_start(
                        out=kuT_sb[:],
                        in_=kuT.rearrange("(c p) e -> p c e", p=128))
                    nc.sync.dma_start(
                        out=vdT_sb[:],
                        in_=vdT.rearrange("(c p) r -> p c r", p=128))
                    nc.scalar.dma_start(
                        out=vuT_sb[:],
                        in_=vuT.rearrange("(c p) e -> p c e", p=128))
                    nc.sync.dma_start(
                        out=qdT_sb[:],
                        in_=qdT.rearrange("(c p) r -> p c r", p=128))
                    nc.scalar.dma_start(
                        out=quT_sb[:],
                        in_=quT.rearrange("(c p) e -> p c e", p=128))

                    def down_proj(xfm, dwT, dst, dst_sl, on_scalar=False):
                        # dst[:, rb, dst_sl] = (W_down @ x^T) feature-major
                        for rb in range(RC):
                            ps = dpsum.tile([128, CH], F32, tag="dps")
                            for ec in range(EC):
                                nc.tensor.matmul(
                                    ps[:], dwT[:, ec, rb * 128:(rb + 1) * 128],
                                    xfm[:, ec, :],
                                    start=(ec == 0), stop=(ec == EC - 1))
                            if on_scalar:
                                nc.scalar.activation(dst[:, rb, dst_sl],
                                                     ps[:], Copy)
                            else:
                                nc.vector.tensor_copy(dst[:, rb, dst_sl],
                                                      ps[:])

                    # --- 1a: k/v down+up, elu, kv accumulation (first, so
                    # the AllReduce overlaps the q down-projection) ---
                    with (
                        tc.tile_pool(name="dd", bufs=2) as ddp,
                        tc.tile_pool(name="upsum", bufs=2,
                                     space="PSUM") as upsum,
                        tc.tile_pool(name="ktm", bufs=2) as ktmp,
                        tc.tile_pool(name="vtm", bufs=2) as vtmp,
                        tc.tile_pool(name="elu1", bufs=2) as elu1,
                        tc.tile_pool(name="kvps", bufs=4,
                                     space="PSUM") as kvps,
                    ):

                        def up_k(dd, ktm):
                            for tb in range(TB):
                                for fc in range(FC5):
                                    ps = upsum.tile([128, 512], F32,
                                                    tag="ups")
                                    for rc in range(RC):
                                        nc.tensor.matmul(
                                            ps[:],
                                            dd[:, rc,
                                               tb * 128:(tb + 1) * 128],
                                            kuT_sb[:, rc,
                                                   fc * 512:(fc + 1) * 512],
                                            start=(rc == 0), stop=False)
                                    nc.tensor.matmul(
                                        ps[:], ones_row[:, 0:128],
                                        ku_brow[:, fc * 512:(fc + 1) * 512],
                                        start=False, stop=True)
                                    m = elu1.tile([128, 512], F32, tag="m")
                                    nc.vector.tensor_scalar_min(
                                        m[:], ps[:], 0.0)
                                    e = elu1.tile([128, 512], F32, tag="e")
                                    nc.scalar.activation(e[:], m[:], Exp)
                                    r = elu1.tile([128, 512], F32, tag="r")
                                    nc.scalar.activation(r[:], ps[:], Relu)
                                    dsl = ktm[:, tb,
                                              fc * hpf:(fc + 1) * hpf, 0:D]
                                    nc.vector.tensor_add(
                                        dsl,
                                        e[:].rearrange("p (h f) -> p h f",
                                                       h=hpf),
                                        r[:].rearrange("p (h f) -> p h f",
                                                       h=hpf))

                        def up_v(dd, vtm):
                            for tb in range(TB):
                                for fc in range(FC5):
                                    ps = upsum.tile([128, 512], F32,
                                                    tag="ups")
                                    for rc in range(RC):
                                        nc.tensor.matmul(
                                            ps[:],
                                            dd[:, rc,
                                               tb * 128:(tb + 1) * 128],
                                            vuT_sb[:, rc,
                                                   fc * 512:(fc + 1) * 512],
                                            start=(rc == 0),
                                            stop=(rc == RC - 1))
                                    dsl = vtm[:, tb,
                                              fc * hpf:(fc + 1) * hpf, 0:D]
                                    bsl = vu_bcast[:, fc * 512:(fc + 1) * 512]
                                    nc.vector.tensor_add(
                                        dsl,
                                        ps[:].rearrange("p (h f) -> p h f",
                                                        h=hpf),
                                        bsl.rearrange("p (h f) -> p h f",
                                                      h=hpf))

                        for c in range(NCH):
                            xfm = xk0 if c == 0 else load_x_fm(xk, c)
                            ddk = ddp.tile([128, RC, CH], BF16, tag="dd")
                            down_proj(xfm, kdT_sb, ddk, slice(None),
                                      on_scalar=True)
                            ktm = ktmp.tile([128, TB, H, D + 1], BF16,
                                            tag="ktm")
                            up_k(ddk, ktm)

                            xfm = load_x_fm(xv, c)
                            ddv = ddp.tile([128, RC, CH], BF16, tag="dd")
                            down_proj(xfm, vdT_sb, ddv, slice(None),
                                      on_scalar=True)
                            vtm = vtmp.tile([128, TB, H, D + 1], BF16,
                                            tag="vtm")
                            nc.vector.memset(vtm[:, :, :, D:D + 1], 1.0)
                            up_v(ddv, vtm)

                            if debug_dump and c == 0:
                                nc.gpsimd.dma_start(out=dbg["ktm"][:],
                                                    in_=ktm[:])
                                nc.gpsimd.dma_start(out=dbg["vtm"][:],
                                                    in_=vtm[:])

                            for hp in range(HP):
                                pkv = kvps.tile([128, D + 1], F32, tag="kvps")
                                for tb in range(TB):
                                    nc.tensor.matmul(
                                        pkv[0:64, :],
                                        ktm[:, tb, 2 * hp, 0:D],
                                        vtm[:, tb, 2 * hp, 0:D + 1],
                                        start=(tb == 0), stop=(tb == TB - 1))
                                for tb in range(TB):
                                    nc.tensor.matmul(
                                        pkv[64:128, :],
                                        ktm[:, tb, 2 * hp + 1, 0:D],
                                        vtm[:, tb, 2 * hp + 1, 0:D + 1],
                                        start=(tb == 0), stop=(tb == TB - 1),
                                        tile_position=(0, 64))
                                acc_sl = kv_acc[:, hp, :]
                                if c == 0:
                                    nc.vector.tensor_copy(acc_sl, pkv[:])
                                else:
                                    nc.vector.tensor_add(acc_sl, acc_sl,
                                                         pkv[:])
                        if debug_dump:
                            nc.sync.dma_start(out=dbg["kvacc"][:],
                                              in_=kv_acc[:])

                    # ------- AllReduce kv across the batch pair (overlaps
                    # the q down-projection below) -------
                    nc.sync.dma_start(out=cc_in[:], in_=kv_acc[:])
                    nc.gpsimd.collective_compute(
                        "AllReduce", mybir.AluOpType.add,
                        ins=[cc_in[:]], outs=[cc_out[:]],
                        replica_groups=groups)

                    # --- 1b: q down-projection ---
                    for c in range(NCH):
                        xfm = load_x_fm(xq, c)
                        down_proj(xfm, qdT_sb, qd_all, bass.ds(c * CH, CH))
                    if debug_dump:
                        nc.gpsimd.dma_start(out=dbg["qd"][:], in_=qd_all[:])

            # ---------------- Phase 2 ----------------
            with (
                tc.tile_pool(name="w2", bufs=1) as w2,
                tc.tile_pool(name="kvx", bufs=1) as kvx,
                tc.tile_pool(name="qps", bufs=2, space="PSUM") as qps,
                tc.tile_pool(name="denp", bufs=1, space="PSUM") as denp,
                tc.tile_pool(name="ndp", bufs=2, space="PSUM") as ndp,
                tc.tile_pool(name="bcp", bufs=1, space="PSUM") as bcp,
                tc.tile_pool(name="yps", bufs=2, space="PSUM") as yps,
                tc.tile_pool(name="qfm", bufs=2) as qfmp,
                tc.tile_pool(name="att", bufs=2) as attp,
                tc.tile_pool(name="rec", bufs=2) as recp,
                tc.tile_pool(name="elu2", bufs=2) as elu2,
                tc.tile_pool(name="ysb", bufs=2) as ysbp,
            ):
                owT_sb = w2.tile([128, EC, E], BF16, tag="owT")
                nc.sync.dma_start(
                    out=owT_sb[:], in_=owT.rearrange("(c p) e -> p c e",
                                                     p=128))

                kv_red = kvx.tile([128, HP, D + 1], F32, tag="kvred")
                nc.sync.dma_start(out=kv_red[:], in_=cc_out[:])
                kv_ext = kvx.tile([128, HP, D + 1], BF16, tag="kvext")
                nc.vector.tensor_copy(kv_ext[:], kv_red[:])
                if debug_dump:
                    nc.sync.dma_start(out=dbg["kvred"][:], in_=kv_red[:])

                # KSmat[p, ec, h] = k_sum[h, p - 64*(h%2)] for h//2==ec else 0
                KSmat = kvx.tile([128, EC, 16], BF16, tag="ksmat")
                nc.vector.memset(KSmat[:], 0.0)
                for h in range(H):
                    base = 64 * (h % 2)
                    nc.vector.tensor_copy(
                        KSmat[base:base + 64, h // 2, h:h + 1],
                        kv_ext[base:base + 64, h // 2, D:D + 1])

                for c in range(NCH):
                    # q up-projection (feature-major) + bias + elu+1
                    qfm = qfmp.tile([128, EC, CH], BF16, tag="qfm")
                    for fc in range(EC):
                        ps = qps.tile([128, CH], F32, tag="qps")
                        for rc in range(RC):
                            nc.tensor.matmul(
                                ps[:], quT_sb[:, rc, fc * 128:(fc + 1) * 128],
                                qd_all[:, rc, bass.ds(c * CH, CH)],
                                start=(rc == 0), stop=(rc == RC - 1))
                        m = elu2.tile([128, CH], F32, tag="m2")
                        nc.vector.tensor_scalar(
                            m[:], ps[:], qu_bc[:, fc:fc + 1], 0.0,
                            op0=mybir.AluOpType.add, op1=mybir.AluOpType.min)
                        e = elu2.tile([128, CH], F32, tag="e2")
                        nc.scalar.activation(e[:], m[:], Exp)
                        r = elu2.tile([128, CH], F32, tag="r2")
                        nc.scalar.activation(r[:], ps[:], Relu,
                                             bias=qu_bc[:, fc:fc + 1])
                        nc.vector.tensor_add(qfm[:, fc, :], e[:], r[:])

                    # denominators for all 16 heads at once
                    den = denp.tile([16, CH], F32, tag="den")
                    for ec in range(EC):
                        nc.tensor.matmul(den[:], KSmat[:, ec, :],
                                         qfm[:, ec, :],
                                         start=(ec == 0), stop=(ec == EC - 1))
                    rec = recp.tile([16, CH], F32, tag="rec")
                    nc.vector.reciprocal_approx_fast(rec[:], den[:])
                    recb = recp.tile([16, CH], BF16, tag="recb")
                    nc.vector.tensor_copy(recb[:], rec[:])
                    if debug_dump and c == 0:
                        dent = recp.tile([16, CH], F32, tag="dent")
                        nc.vector.tensor_copy(dent[:], den[:])
                        nc.sync.dma_start(out=dbg["den"][:], in_=dent[:])
                        nc.sync.dma_start(out=dbg["rec"][:], in_=rec[:])
                        nc.gpsimd.dma_start(out=dbg["qfm"][:], in_=qfm[:])

                    # attention per head pair: packed num matmuls + bcast mul
                    att = attp.tile([128, EC, CH], BF16, tag="att")
                    for hp in range(HP):
                        nd = ndp.tile([128, CH], F32, tag="nd")
                        nc.tensor.matmul(nd[0:64, :], kv_ext[0:64, hp, 0:D],
                                         qfm[0:64, hp, :],
                                         start=True, stop=True)
                        nc.tensor.matmul(nd[64:128, :],
                                         kv_ext[64:128, hp, 0:D],
                                         qfm[64:128, hp, :],
                                         start=True, stop=True,
                                         tile_position=(64, 64))
                        bc = bcp.tile([128, CH], F32, tag="bc")
                        nc.tensor.matmul(bc[:], ppat[:, hp, :], recb[:],
                                         start=True, stop=True)
                        bcs = elu2.tile([128, CH], F32, tag="bcs")
                        nc.scalar.activation(bcs[:], bc[:], Copy)
                        nc.vector.tensor_mul(att[:, hp, :], nd[:], bcs[:])
                        if debug_dump and c == 0 and hp == 0:
                            ndt = elu2.tile([128, CH], F32, tag="ndt")
                            nc.vector.tensor_copy(ndt[:], nd[:])
                            nc.sync.dma_start(out=dbg["nd0"][:], in_=ndt[:])
                            nc.sync.dma_start(out=dbg["bc0"][:], in_=bcs[:])

                    if debug_dump and c == 0:
                        nc.gpsimd.dma_start(out=dbg["att"][:], in_=att[:])

                    # output projection (token-major) + bias
                    for tb in range(TB):
                        ysb = ysbp.tile([128, E], F32, tag="ysb")
                        for fo in range(FC5):
                            py = yps.tile([128, 512], F32, tag="yps")
                            for ec in range(EC):
                                nc.tensor.matmul(
                                    py[:],
                                    att[:, ec, tb * 128:(tb + 1) * 128],
                                    owT_sb[:, ec, fo * 512:(fo + 1) * 512],
                                    start=(ec == 0), stop=(ec == EC - 1))
                            nc.vector.tensor_add(
                                ysb[:, fo * 512:(fo + 1) * 512], py[:],
                                ou_bcast[:, fo * 512:(fo + 1) * 512])
                        r0 = c * CH + tb * 128
                        nc.sync.dma_start(out=y[r0:r0 + 128, :], in_=ysb[:])

    nc.compile()
    return nc


_NC_CACHE = {}


def _get_nc(T, n_cores, groups):
    key = (T, n_cores, tuple(tuple(g) for g in groups))
    if key not in _NC_CACHE:
        _NC_CACHE[key] = build_nc(T, n_cores, groups)
    return _NC_CACHE[key]


def _make_in_maps(inputs):
    bf = ml_dtypes.bfloat16
    query = np.asarray(inputs["query"], dtype=np.float32)
    key = np.asarray(inputs["key"], dtype=np.float32)
    value = np.asarray(inputs["value"], dtype=np.float32)

    weights = {
        "qdT": np.asarray(inputs["qd_w"], np.float32).T.astype(bf),
        "kdT": np.asarray(inputs["kd_w"], np.float32).T.astype(bf),
        "vdT": np.asarray(inputs["vd_w"], np.float32).T.astype(bf),
        "quT": np.asarray(inputs["qu_w"], np.float32).T.astype(bf),
        "kuT": np.asarray(inputs["ku_w"], np.float32).T.astype(bf),
        "vuT": np.asarray(inputs["vu_w"], np.float32).T.astype(bf),
        "owT": np.asarray(inputs["out_w"], np.float32).T.astype(bf),
        "qu_b": np.asarray(inputs["qu_b"], np.float32),
        "ku_b": np.asarray(inputs["ku_b"], np.float32).astype(bf),
        "vu_b": np.asarray(inputs["vu_b"], np.float32).astype(bf),
        "out_b": np.asarray(inputs["out_b"], np.float32).astype(bf),
    }
    HP = H // 2
    ppat = np.zeros((16, HP * 128), dtype=np.float32)
    for hp in range(HP):
        ppat[2 * hp, hp * 128:hp * 128 + 64] = 1.0
        ppat[2 * hp + 1, hp * 128 + 64:hp * 128 + 128] = 1.0
    weights["ppat"] = ppat.astype(bf)

    half = S // 2
    in_maps = []
    for c in range(N_CORES):
        bi, hi = c // 2, c % 2
        sl = slice(hi * half, (hi + 1) * half)
        m = {
            "xq": query[bi, sl].T.astype(bf),
            "xk": key[bi, sl].T.astype(bf),
            "xv": value[bi, sl].T.astype(bf),
        }
        m.update(weights)
        in_maps.append(m)
    return in_maps


def kernel(**inputs):
    b, s, e = np.asarray(inputs["query"]).shape
    assert (b, s, e) == (B, S, E)

    T = B * S // N_CORES  # 2048 tokens per core
    half = S // 2
    groups = [[0, 1], [2, 3], [4, 5], [6, 7]]
    nc = _get_nc(T, N_CORES, groups)

    in_maps = _make_in_maps(inputs)
    res = run_bass_kernel_spmd(nc, in_maps, list(range(N_CORES)))

    out = np.empty((B, S, E), dtype=np.float32)
    for c in range(N_CORES):
        bi, hi = c // 2, c % 2
        out[bi, hi * half:(hi + 1) * half] = res.results[c]["y"]
    return out
